# revision 1
# baseline (speedup 1.0000x reference)
"""ChildSum TreeLSTM + attention, 8-core SPMD Trainium2 kernel (v2).

Sharding: mem_dim 1024 in 8 slices of 128 (core j owns slice j); the 512
attention rows in 8 groups of 64. Two XOR all-to-all syncs per step (7
single-dest remote_dma_broadcast preps each, self contribution added in the
receive reduction).

v2 critical-path design (vs the v1 baseline kept below for reference):
  - SWDGE descriptor preps are HOISTED off the critical path: sync1(t+1)
    preps enqueue right after trig2(t), sync2(t) preps after trig1(t); only
    trigger_dma (gated on a payload sem_inc fence) sits on the path.
  - Gate GEMV split: psA1 = Wr^T(csum + h_new_full) runs hidden under the
    attn+sync2 window; on the path only psA2 = (H Wr)^T e (16 MMs, K=e) and
    gates = ACT(psA1 - psA2/Z + xproj[t]).  g0 = Wr^T csum is folded into
    xproj (removed again at t=0 where h(-1)=0).
  - exp via sigmoid: e = sig(s) * recip(sig(-s)) keeps ACT on one table set
    (sigmoid+tanh) -> zero ~2.7us ACT table reloads per step.
  - attention tanh is ONE ACT [128,512]; the w bias is pre-added with a
    single DVE using a stride-0 broadcast AP.
  - numer moved post-sync2 and sharded: sync2 carries only the one-hot e
    columns [128,4]; each core computes its own numer slice (4 MMs) and its
    own h_att slice; host gathers the 8 per-core hout [128,T] outputs.
  - h_att/houtbuf writes are off-path; one final 128KB DMA.
  - ones matrix is -1 so 1/(-Z) fuses the subtractions into
    scalar_tensor_tensor ops.
  - bf16 stationary weights (wrec/hwr/w1t/hTj, FWL 2x weight load), fp32
    PSUM accumulate; rhs vectors cast per step. l2 rel vs fp32 ref: 7.8e-6.

Host prep does layout only (transpose/slice/cast) - zero FLOPs.
"""
import dataclasses
import numpy as np
from contextlib import ExitStack

import concourse.bass as bass
import concourse.tile as tile
from concourse import bacc, mybir
from concourse.bass import create_sync_update
from concourse.tile_rust import add_dep_helper

F32 = mybir.dt.float32
AF = mybir.ActivationFunctionType
N_CORES = 8
MEM = 1024
IN_DIM = 1024
MROWS = 512          # attention rows
KC = MEM // 128      # 8 column chunks
RPC = MROWS // N_CORES  # 64 attention rows per core

# Sems incremented by remote cores / DMA completion that the Tile scheduling
# pass (single-core sim, no_exec) can never see. Pre-satisfied there only;
# the runtime NEFF keeps the real waits.
_EXTERNAL_SEMS: list = []
_OrigCoreSim = tile.CoreSim


class _SchedCoreSim(_OrigCoreSim):
    def __init__(self, *a, **kw):
        super().__init__(*a, **kw)
        for sem in _EXTERNAL_SEMS:
            self.update_semaphore(create_sync_update(sem, 1 << 22))


tile.CoreSim = _SchedCoreSim


def prep_in_maps(inputs: dict, T: int, wdt_np=np.float32) -> list[dict]:
    """Host-side layout-only prep of per-core input maps."""
    X = np.asarray(inputs["inputs"], np.float32).reshape(T, IN_DIM)
    H = np.asarray(inputs["hiddn_state_mat"], np.float32)
    W_ioux = np.asarray(inputs["W_ioux"], np.float32)
    W_iouh = np.asarray(inputs["W_iouh"], np.float32)
    W_fx = np.asarray(inputs["W_fx"], np.float32)
    W_fh = np.asarray(inputs["W_fh"], np.float32)
    Wa = np.asarray(inputs["Wa"], np.float32).reshape(MEM)
    W_attnh = np.asarray(inputs["W_attnh"], np.float32)
    b_iou = (np.asarray(inputs["b_ioux"], np.float32)
             + np.asarray(inputs["b_iouh"], np.float32))
    b_f = (np.asarray(inputs["b_fx"], np.float32)
           + np.asarray(inputs["b_fh"], np.float32))
    b_attnh = np.asarray(inputs["b_attnh"], np.float32)

    W1 = W_attnh[:MEM]
    W2 = W_attnh[MEM:]

    # replicated tensors
    xT_l = X.T.reshape(KC, 128, T).transpose(1, 0, 2).reshape(128, KC * T)
    xT_l = np.ascontiguousarray(xT_l)
    # w2t tile(c,k) = W2[128k+a, 128c+b] -> [128, 64*128]
    w2t = np.zeros((128, KC * KC * 128), np.float32)
    for c in range(KC):
        for k in range(KC):
            w2t[:, (c * KC + k) * 128:(c * KC + k + 1) * 128] = \
                W2[128 * k:128 * (k + 1), 128 * c:128 * (c + 1)]
    hT_l = np.ascontiguousarray(
        H.T.reshape(KC, 128, MROWS).transpose(1, 0, 2).reshape(128, KC * MROWS))
    wa_l = np.ascontiguousarray(Wa.reshape(KC, 128).T)
    b2_l = np.ascontiguousarray(b_attnh.reshape(KC, 128).T)

    gate_w = [W_iouh[:, 0:MEM], W_iouh[:, MEM:2 * MEM], W_iouh[:, 2 * MEM:], W_fh]
    gate_wx = [W_ioux[:, 0:MEM], W_ioux[:, MEM:2 * MEM], W_ioux[:, 2 * MEM:], W_fx]
    gate_b = [b_iou[0:MEM], b_iou[MEM:2 * MEM], b_iou[2 * MEM:], b_f]

    maps = []
    for j in range(N_CORES):
        wrec = np.zeros((128, 4 * KC * 128), np.float32)
        wx = np.zeros((128, 4 * KC * 128), np.float32)
        for g in range(4):
            for k in range(KC):
                sl = np.s_[:, (g * KC + k) * 128:(g * KC + k + 1) * 128]
                wrec[sl] = gate_w[g][128 * k:128 * (k + 1), 128 * j:128 * (j + 1)]
                wx[sl] = gate_wx[g][128 * k:128 * (k + 1), 128 * j:128 * (j + 1)]
        w1t = np.zeros((128, KC * 128), np.float32)
        for m in range(KC):
            w1t[:, m * 128:(m + 1) * 128] = \
                W1[128 * j:128 * (j + 1), 128 * m:128 * (m + 1)]
        bias_x = np.stack([gate_b[g][128 * j:128 * (j + 1)] for g in range(4)], axis=1)
        mask = np.zeros((128, KC), np.float32)
        mask[:, j] = 1.0
        Hown = H[RPC * j:RPC * (j + 1)]            # [64, 1024]
        hTown_l = np.ascontiguousarray(
            Hown.T.reshape(KC, 128, RPC).transpose(1, 0, 2).reshape(128, KC * RPC))
        maps.append({
            "xT": xT_l, "wx": wx,
            "wrec": wrec.astype(wdt_np), "w1t": w1t.astype(wdt_np),
            "w2t": w2t, "hT": hT_l, "hTown": hTown_l,
            "hrows": np.ascontiguousarray(H[RPC * j:RPC * (j + 1)]).astype(wdt_np),
            "wa": wa_l, "bias_x": np.ascontiguousarray(bias_x),
            "bias2": b2_l, "mask": mask,
        })
    return maps


def postprocess(hout_core0: np.ndarray, T: int) -> np.ndarray:
    # hout [T, 128, KC]: [t, p, c] = h_att_t[128c + p]
    return np.ascontiguousarray(
        hout_core0.transpose(0, 2, 1).reshape(T, MEM)).astype(np.float32)




# ============================================================================
# v2: hoisted SWDGE preps (triggers only on the critical path), gate GEMV
# split A1 = Wr^T(csum+h_new) [hidden under attn+sync2] + A2 = (H Wr)^T e
# [16 MMs on path], exp via sigma(x)/sigma(-x) (no ACT table switches),
# h_att/output assembly off-path, hout batched into one final DMA.
# ============================================================================
def build_nc_v2(T: int, t_run: int | None = None, no_comm: bool = False,
                bf16: bool = False):
    del _EXTERNAL_SEMS[:]
    wdt = mybir.dt.bfloat16 if bf16 else F32
    nc = bacc.Bacc(dynamic_dma_scratch_size=32768)
    TR = t_run if t_run is not None else T

    dp = lambda n, s_, dt=F32: nc.declare_dram_parameter(n, s_, dt, isOutput=False)
    xT = dp("xT", [128, KC * T])            # xT[p, T*k+t] = X[t, 128k+p]
    wx = dp("wx", [128, 4 * KC * 128])      # tile(g,k): Wx_g[128k+a, 128j+b]
    wrec = dp("wrec", [128, 4 * KC * 128])  # tile(g,k): Wr_g[128k+a, 128j+b]
    w1t = dp("w1t", [128, KC * 128])        # tile m: W1[128j+a, 128m+b]
    w2t = dp("w2t", [128, KC * KC * 128])   # tile(c,k): W2[128k+a, 128c+b]
    hTs = dp("hT", [128, KC * MROWS])       # hT[p, 512m+i] = H[i, 128m+p]
    hTown = dp("hTown", [128, KC * RPC])    # hTown[p, 64k+i] = H[64j+i, 128k+p]
    hTj = dp("hTj", [128, MROWS])           # tile E: H[128E+i, 128j+p] (K=e rows)
    wa = dp("wa", [128, KC])                # wa[p,c] = Wa[128c+p]
    bias_x = dp("bias_x", [128, 4])         # per-gate bias for own slice
    bias2 = dp("bias2", [128, KC])          # b_attnh column form
    mask = dp("mask", [128, KC])            # one-hot col own_core
    emask = dp("emask", [128, 4])           # one-hot e-placement [rows (j%2)*64, col j//2]
    dup2 = dp("dup2", [RPC, 128])           # [I64 | I64]: dup e to both halves
    hout = nc.declare_dram_parameter("hout", [128, T], F32, isOutput=True)

    with tile.TileContext(nc) as tc, ExitStack() as ctx:
        paysem = ctx.enter_context(nc.semaphore("paysem"))
        sem1 = ctx.enter_context(nc.semaphore("rdma_sem1"))
        sem2 = ctx.enter_context(nc.semaphore("rdma_sem2"))
        lsem1 = ctx.enter_context(nc.semaphore("rdma_lsem1"))
        lsem2 = ctx.enter_context(nc.semaphore("rdma_lsem2"))
        _EXTERNAL_SEMS.extend([sem1, sem2, lsem1, lsem2])

        comm = ctx.enter_context(tc.tile_pool(name="comm", bufs=1))
        pay1 = [comm.tile([128, 16], F32, name=f"pay1_{p}", tag=f"pay1_{p}") for p in range(2)]
        rec1 = [comm.tile([128, 112], F32, name=f"rec1_{p}", tag=f"rec1_{p}") for p in range(2)]
        pay2 = [comm.tile([128, 4], F32, name=f"pay2_{p}", tag=f"pay2_{p}") for p in range(2)]
        rec2 = [comm.tile([128, 28], F32, name=f"rec2_{p}", tag=f"rec2_{p}") for p in range(2)]

        const = ctx.enter_context(tc.tile_pool(name="const", bufs=1))
        wrec_sb = const.tile([128, 4 * KC * 128], F32, tag="wrec")
        w1t_sb = const.tile([128, KC * 128], F32, tag="w1t")
        hTj_sb = const.tile([128, MROWS], F32, tag="hTj")
        csumj_sb = const.tile([128, 1], F32, tag="csumj")
        wa_sb = const.tile([128, KC], F32, tag="wa")
        hw2T_sb = const.tile([128, KC * RPC], F32, tag="hw2T")
        hwr_sb = const.tile([128, 16 * 128], wdt, tag="hwr")   # tile(g,E): HWr[128E+e, g-col]
        xproj_sb = const.tile([128, 4 * T], F32, tag="xproj")
        xprojt_sb = const.tile([128, 4 * T], F32, tag="xprojt")
        csum_sb = const.tile([128, KC], F32, tag="csum")
        ones_sb = const.tile([128, 128], F32, tag="ones")
        mask_sb = const.tile([128, KC], F32, tag="mask")
        emask_sb = const.tile([128, 4], F32, tag="emask")
        dup2_sb = const.tile([RPC, 128], F32, tag="dup2")
        zero4_sb = const.tile([128, 4], F32, tag="zero4")
        houtbuf = const.tile([128, T], F32, tag="houtbuf")
        if bf16:
            wrec_w = const.tile([128, 4 * KC * 128], wdt, tag="wrec_w")
            hTj_w = const.tile([128, MROWS], wdt, tag="hTj_w")

        nc.sync.dma_start(wrec_sb[:, :], wrec.ap())
        nc.sync.dma_start(w1t_sb[:, :], w1t.ap())
        nc.sync.dma_start(hTj_sb[:, :], hTj.ap())
        nc.sync.dma_start(wa_sb[:, :], wa.ap())
        nc.sync.dma_start(mask_sb[:, :], mask.ap())
        nc.sync.dma_start(emask_sb[:, :], emask.ap())
        nc.sync.dma_start(dup2_sb[:, :], dup2.ap())
        nc.vector.memset(ones_sb[:, :], -1.0)  # psZ = -Z so rz = -1/Z (fused adds)
        if bf16:
            nc.vector.tensor_copy(wrec_w[:, :], wrec_sb[:, :])
            nc.vector.tensor_copy(hTj_w[:, :], hTj_sb[:, :])
        else:
            wrec_w, hTj_w = wrec_sb, hTj_sb
        nc.vector.memset(zero4_sb[:, :], 0.0)
        nc.vector.memset(houtbuf[:, :], 0.0)
        for p in range(2):
            nc.vector.memset(pay2[p][:, :], 0.0)
        if no_comm:
            for p in range(2):
                nc.vector.memset(rec1[p][:, :], 0.01)
                nc.vector.memset(rec2[p][:, :], 0.01)

        # ---------- device precompute ----------
        with tc.tile_pool(name="pre", bufs=1) as pre, \
             tc.tile_pool(name="prepsum", bufs=1, space="PSUM") as pps:
            xT_sb = pre.tile([128, KC * T], F32, tag="xT")
            wx_sb = pre.tile([128, 4 * KC * 128], F32, tag="wx")
            w2t_sb = pre.tile([128, KC * KC * 128], F32, tag="w2t")
            hT_sb = pre.tile([128, KC * MROWS], F32, tag="hT")
            hTown_sb = pre.tile([128, KC * RPC], F32, tag="hTown")
            bx_sb = pre.tile([128, 4], F32, tag="bias_x")
            b2_sb = pre.tile([128, KC], F32, tag="bias2")
            nc.sync.dma_start(xT_sb[:, :], xT.ap())
            nc.sync.dma_start(wx_sb[:, :], wx.ap())
            nc.sync.dma_start(w2t_sb[:, :], w2t.ap())
            nc.sync.dma_start(hT_sb[:, :], hTs.ap())
            nc.sync.dma_start(hTown_sb[:, :], hTown.ap())
            nc.sync.dma_start(bx_sb[:, :], bias_x.ap())
            nc.sync.dma_start(b2_sb[:, :], bias2.ap())

            # colsumH column form
            for m in range(KC):
                nc.vector.reduce_sum(
                    csum_sb[:, m:m + 1],
                    hT_sb[:, m * MROWS:(m + 1) * MROWS],
                    axis=mybir.AxisListType.X,
                )

            # own column of csum (one-hot mask over the 8 chunk cols)
            cm = pre.tile([128, KC], F32, tag="cmask")
            nc.vector.tensor_mul(cm[:, :], csum_sb[:, :], mask_sb[:, :])
            nc.vector.reduce_sum(csumj_sb[:, :], cm[:, :], axis=mybir.AxisListType.X)

            # g0 = Wr^T csum (own 4 gate cols); combined bias = bias_x + g0
            psg0 = pps.tile([128, 4], F32, tag="ps_g0")
            for g in range(4):
                for k in range(KC):
                    nc.tensor.matmul(
                        psg0[:, g:g + 1],
                        wrec_sb[:, (g * KC + k) * 128:(g * KC + k + 1) * 128],
                        csum_sb[:, k:k + 1],
                        start=(k == 0), stop=(k == KC - 1),
                    )
            biasc = pre.tile([128, 4], F32, tag="biasc")
            nc.vector.tensor_add(biasc[:, :], psg0[:, :], bx_sb[:, :])

            # xproj[g]: [128, T] = sum_k Wx_g[k]^T @ xT[k] (+ combined bias)
            for g in range(4):
                ps = pps.tile([128, T], F32, tag="ps_x")
                for k in range(KC):
                    nc.tensor.matmul(
                        ps[:, :],
                        wx_sb[:, (g * KC + k) * 128:(g * KC + k + 1) * 128],
                        xT_sb[:, k * T:(k + 1) * T],
                        start=(k == 0), stop=(k == KC - 1),
                    )
                nc.vector.tensor_scalar_add(
                    xproj_sb[:, g * T:(g + 1) * T], ps[:, :], biasc[:, g:g + 1]
                )
                # h(-1) = 0: no csum term at t=0, so remove the folded g0
                nc.vector.tensor_sub(
                    xproj_sb[:, g * T:g * T + 1],
                    xproj_sb[:, g * T:g * T + 1], psg0[:, g:g + 1]
                )
            # repack g-major -> t-major: xprojt[:, 4t+g] = xproj[:, g*T+t]
            nc.vector.tensor_copy(
                xprojt_sb[:, :].rearrange("p (t g) -> p t g", g=4),
                xproj_sb[:, :].rearrange("p (g t) -> p t g", g=4),
            )

            # hw2T c-chunk: [128, 64] = sum_k W2[c,k]^T @ hTown[k]  (+ b2)
            for c in range(KC):
                ps2 = pps.tile([128, RPC], F32, tag="ps_h")
                for k in range(KC):
                    nc.tensor.matmul(
                        ps2[:, :],
                        w2t_sb[:, (c * KC + k) * 128:(c * KC + k + 1) * 128],
                        hTown_sb[:, k * RPC:(k + 1) * RPC],
                        start=(k == 0), stop=(k == KC - 1),
                    )
                nc.vector.tensor_scalar_add(
                    hw2T_sb[:, c * RPC:(c + 1) * RPC], ps2[:, :], b2_sb[:, c:c + 1]
                )

            # hwr tile(g,E) = HWr[128E:128E+128, own gate col g] = sum_k H[E,k] Wr[k,g]
            for g in range(4):
                for E in range(4):
                    ps3 = pps.tile([128, 128], F32, tag="ps_hwr")
                    for k in range(KC):
                        nc.tensor.matmul(
                            ps3[:, :],
                            hT_sb[:, k * MROWS + 128 * E:k * MROWS + 128 * (E + 1)],
                            wrec_sb[:, (g * KC + k) * 128:(g * KC + k + 1) * 128],
                            start=(k == 0), stop=(k == KC - 1),
                        )
                    nc.vector.tensor_copy(
                        hwr_sb[:, (g * 4 + E) * 128:(g * 4 + E + 1) * 128], ps3[:, :]
                    )

        # ---------- state & per-step pools ----------
        sp = ctx.enter_context(tc.tile_pool(name="step", bufs=2))
        psp = ctx.enter_context(tc.tile_pool(name="spsum", bufs=1, space="PSUM"))

        chain = [None]
        nprep = [0]

        def preps(pay, recs, rsem, lsem):
            prev = chain[0]
            for k in range(1, N_CORES):
                rdests = [None] * N_CORES
                rdests[k] = (0, k)
                inst = nc.gpsimd.remote_dma_broadcast(
                    out_ap=recs(k), in_ap=pay, remote_sem=rsem, local_sem=lsem,
                    rdests=rdests,
                )
                if prev is not None:
                    add_dep_helper(inst.ins, prev, False, "swdge ring order")
                prev = inst.ins
            chain[0] = prev
            nprep[0] += 1

        npay = [0]

        lastfence = [None]

        def trig(payload_insts):
            # count=None: Tile manages prep->trigger HW ordering. Payload gate:
            # the DMA reads the pay tile only at fire time, and the preps were
            # emitted before this step's payload writes. DVE is in-order, so an
            # explicit sem_inc ordered after the payload writes proves them
            # complete; the trigger waits on paysem. A dedicated sem avoids the
            # engine-clock granularity deadlock (trigger must not wait on DVE
            # ticks past the sem1/sem2 waits).
            fence = nc.vector.sem_inc(paysem, 1)
            for pi in payload_insts:
                add_dep_helper(fence.ins, pi.ins, False, "fence after payload")
            if lastfence[0] is not None:
                add_dep_helper(fence.ins, lastfence[0].ins, False, "fence order")
            lastfence[0] = fence
            npay[0] += 1
            w = nc.gpsimd.wait_ge(paysem, npay[0])
            add_dep_helper(w.ins, chain[0], False, "swdge ring order")
            tg = nc.gpsimd.trigger_dma(count=None)
            add_dep_helper(tg.ins, w.ins, False, "swdge ring order")
            chain[0] = tg.ins

        gates3 = comm.tile([128, 1], F32, tag="gates3")
        ccol = sp.tile([128, 1], F32, tag="ccol")
        nc.vector.memset(ccol[:, :], 0.0)
        hnew_full_prev = None
        h_new_prev = None
        a1sb_prev = None
        psA1 = psp.tile([128, 4], F32, tag="psA1")

        for t in range(TR):
            par = t & 1
            parp = (t - 1) & 1

            if t == 0 and not no_comm:
                preps(pay1[0][:, 0:16],
                      lambda k: rec1[0][:, (k - 1) * 16:k * 16], sem1, lsem1)

            # ===== post-sync2 of step t-1: rz, A2, h_att(t-1) =====
            if t >= 1:
                red2 = sp.tile([128, 4], F32, tag="red2")
                if not no_comm:
                    w2_inst = nc.vector.wait_ge(sem2, 14 * t)
                    add_dep_helper(w2_inst.ins, lastfence[0].ins, False, "wait after fence")
                r2 = rec2[parp][:, :].rearrange("p (s c) -> p c s", s=N_CORES - 1)
                i3 = nc.vector.reduce_sum(red2[:, :], r2[:, :, :], axis=mybir.AxisListType.X)
                nc.vector.tensor_add(red2[:, :], red2[:, :], pay2[parp][:, 0:4])
                if not no_comm:
                    add_dep_helper(i3.ins, w2_inst.ins, False, "gate recv2 on sem2")
                # Z and 1/Z first (psZ ahead of A2 in the PE FIFO)
                zpart = sp.tile([128, 1], F32, tag="zpart")
                nc.vector.reduce_sum(zpart[:, :], red2[:, :], axis=mybir.AxisListType.X)
                psZ = psp.tile([128, 1], F32, tag="psZ")
                nc.tensor.matmul(psZ[:, :], ones_sb[:, :], zpart[:, :], start=True, stop=True)
                rz = sp.tile([128, 1], F32, tag="rz")
                nc.vector.reciprocal(rz[:, :], psZ[:, :])
                # A2 = hwr^T @ ecol (on path)
                if bf16:
                    red2u = sp.tile([128, 4], wdt, tag="red2u")
                    nc.vector.tensor_copy(red2u[:, :], red2[:, :])
                else:
                    red2u = red2
                psA2 = psp.tile([128, 4], F32, tag="psA2")
                for g in (3, 0, 1, 2):
                    for E in range(4):
                        nc.tensor.matmul(
                            psA2[:, g:g + 1],
                            hwr_sb[:, (g * 4 + E) * 128:(g * 4 + E + 1) * 128],
                            red2u[:, E:E + 1],
                            start=(E == 0), stop=(E == 3),
                        )
                    if g == 3:
                        dpre3 = sp.tile([128, 1], F32, tag="dpre3")
                        nc.vector.scalar_tensor_tensor(
                            dpre3[:, :], psA2[:, 3:4], rz[:, :], a1sb_prev[:, 3:4],
                            mybir.AluOpType.mult, mybir.AluOpType.add)
                        nc.scalar.activation(gates3[:, :], dpre3[:, :], AF.Tanh)
                # own slice of numer + h_att(t-1) output (off critical path)
                psNo = psp.tile([128, 1], F32, tag="psNo")
                for E in range(4):
                    nc.tensor.matmul(
                        psNo[:, :], hTj_w[:, 128 * E:128 * (E + 1)], red2u[:, E:E + 1],
                        start=(E == 0), stop=(E == 3),
                    )
                htmp = sp.tile([128, 1], F32, tag="htmp")
                nc.vector.tensor_add(htmp[:, :], h_new_prev[:, :], csumj_sb[:, :])
                nc.vector.scalar_tensor_tensor(
                    houtbuf[:, t - 1:t], psNo[:, :], rz[:, :], htmp[:, :],
                    mybir.AluOpType.mult, mybir.AluOpType.add)

            # ===== gates(t) =====
            # gate order [i, o, f, u]: sigmoid on 0:3; tanh(u) done early (gates3)
            gates = sp.tile([128, 4], F32, tag="gates")
            if t == 0:
                nc.scalar.activation(gates[:, 0:3], xprojt_sb[:, 4 * t:4 * t + 3], AF.Sigmoid)
                nc.scalar.activation(gates3[:, :], xprojt_sb[:, 4 * t + 3:4 * t + 4], AF.Tanh)
            else:
                dpre = sp.tile([128, 3], F32, tag="dpre")
                nc.vector.scalar_tensor_tensor(
                    dpre[:, :], psA2[:, 0:3], rz[:, :], a1sb_prev[:, 0:3],
                    mybir.AluOpType.mult, mybir.AluOpType.add)
                nc.scalar.activation(gates[:, 0:3], dpre[:, :], AF.Sigmoid)
            iu = sp.tile([128, 1], F32, tag="iu")
            nc.vector.tensor_mul(iu[:, :], gates[:, 0:1], gates3[:, :])
            ccol_new = sp.tile([128, 1], F32, tag="ccol")
            nc.vector.scalar_tensor_tensor(
                ccol_new[:, :], ccol[:, :], gates[:, 2:3], iu[:, :],
                mybir.AluOpType.mult, mybir.AluOpType.add)
            ccol = ccol_new
            tanh_c = sp.tile([128, 1], F32, tag="tanh_c")
            nc.scalar.activation(tanh_c[:, :], ccol[:, :], AF.Tanh)
            h_new = sp.tile([128, 1], F32, tag="h_new")
            hn_inst = nc.vector.tensor_mul(h_new[:, :], gates[:, 1:2], tanh_c[:, :])

            # ===== pay1 = [h_new one-hot | W1^T h_new] ; TRIG1 =====
            psW = psp.tile([128, KC], F32, tag="psW")
            for m in range(KC):
                nc.tensor.matmul(
                    psW[:, m:m + 1], w1t_sb[:, m * 128:(m + 1) * 128], h_new[:, :],
                    start=True, stop=True,
                )
            if t >= 2 and not no_comm:
                lw1 = nc.vector.wait_ge(lsem1, 112 * t)
                add_dep_helper(lw1.ins, hn_inst.ins, False, "anchor lsem1 wait")
                add_dep_helper(lw1.ins, lastfence[0].ins, False, "wait after fence")
            mm1 = nc.vector.tensor_scalar_mul(pay1[par][:, 0:8], mask_sb[:, :], h_new[:, :])
            cp1 = nc.vector.tensor_copy(pay1[par][:, 8:16], psW[:, :])
            if t >= 2 and not no_comm:
                add_dep_helper(mm1.ins, lw1.ins, False, "pay1 WAR")
                add_dep_helper(cp1.ins, lw1.ins, False, "pay1 WAR")
            if not no_comm:
                trig([mm1, cp1])
                # enqueue sync2(t) preps now (off-path, overlaps flight1+attn)
                preps(pay2[par][:, 0:4],
                      lambda k: rec2[par][:, (k - 1) * 4:k * 4], sem2, lsem2)
                w1_inst = nc.vector.wait_ge(sem1, 14 * (t + 1))
                add_dep_helper(w1_inst.ins, cp1.ins, False, "anchor sem1 wait")
                add_dep_helper(w1_inst.ins, lastfence[0].ins, False, "wait after fence")

            red1 = sp.tile([128, 16], F32, tag="red1")
            r1 = rec1[par][:, :].rearrange("p (s c) -> p c s", s=N_CORES - 1)
            # w_sum half first: it is the only on-path consumer (attention
            # bias); the h_new half feeds only the off-path psA1 GEMV.
            i1 = nc.vector.reduce_sum(red1[:, 8:16], r1[:, 8:16, :], axis=mybir.AxisListType.X)
            nc.vector.tensor_add(red1[:, 8:16], red1[:, 8:16], pay1[par][:, 8:16])
            ih = nc.vector.reduce_sum(red1[:, 0:8], r1[:, 0:8, :], axis=mybir.AxisListType.X)
            nc.vector.tensor_add(red1[:, 0:8], red1[:, 0:8], pay1[par][:, 0:8])
            if not no_comm:
                add_dep_helper(i1.ins, w1_inst.ins, False, "gate recv1 on sem1")
                add_dep_helper(ih.ins, w1_inst.ins, False, "gate recv1 on sem1")

            # ===== attention =====
            tT = sp.tile([128, KC * RPC], F32, tag="tT")
            tpre = sp.tile([128, KC * RPC], F32, tag="tpre")
            wv = red1[:, 8:16]
            wb = dataclasses.replace(wv, ap=[*wv.ap, [0, RPC]])
            nc.vector.tensor_tensor(
                tpre[:, :].rearrange("p (c i) -> p c i", c=KC),
                hw2T_sb[:, :].rearrange("p (c i) -> p c i", c=KC),
                wb, mybir.AluOpType.add)
            half = KC * RPC // 2
            nc.scalar.activation(tT[:, 0:half], tpre[:, 0:half], AF.Tanh)
            nc.scalar.activation(tT[:, half:], tpre[:, half:], AF.Tanh)
            psS = psp.tile([RPC, 1], F32, tag="psS")
            for c in range(KC):
                nc.tensor.matmul(
                    psS[:, :], tT[:, c * RPC:(c + 1) * RPC], wa_sb[:, c:c + 1],
                    start=(c == 0), stop=(c == KC - 1),
                )
            ep = sp.tile([RPC, 1], F32, tag="ep")
            nc.scalar.activation(ep[:, :], psS[:, :], AF.Sigmoid)
            en = sp.tile([RPC, 1], F32, tag="en")
            nc.scalar.activation(en[:, :], psS[:, :], AF.Sigmoid, scale=-1.0)
            ren = sp.tile([RPC, 1], F32, tag="ren")
            nc.vector.reciprocal(ren[:, :], en[:, :])
            e_own = sp.tile([RPC, 1], F32, tag="e_own")
            ei = nc.vector.tensor_mul(e_own[:, :], ep[:, :], ren[:, :])

            psE = psp.tile([128, 1], F32, tag="psE")
            nc.tensor.matmul(psE[:, :], dup2_sb[:, :], e_own[:, :], start=True, stop=True)

            if t >= 2 and not no_comm:
                lw2 = nc.vector.wait_ge(lsem2, 112 * t)
                add_dep_helper(lw2.ins, ei.ins, False, "anchor lsem2 wait")
                add_dep_helper(lw2.ins, lastfence[0].ins, False, "wait after fence")
            em2 = nc.vector.tensor_scalar_mul(pay2[par][:, 0:4], emask_sb[:, :], psE[:, :])
            if t >= 2 and not no_comm:
                add_dep_helper(em2.ins, lw2.ins, False, "pay2 WAR")
            if not no_comm:
                trig([em2])
                if t + 1 < TR:
                    # enqueue sync1(t+1) preps (off-path, overlaps flight2)
                    preps(pay1[1 - par][:, 0:16],
                          lambda k: rec1[1 - par][:, (k - 1) * 16:k * 16], sem1, lsem1)

            # ===== psA1(t+1) = Wr^T hnew_full  (hidden under attn+sync2) =====
            if t + 1 < TR:
                if bf16:
                    red1u = sp.tile([128, KC], wdt, tag="red1u")
                    nc.vector.tensor_copy(red1u[:, :], red1[:, 0:8])
                else:
                    red1u = None
                for g in range(4):
                    for k in range(KC):
                        nc.tensor.matmul(
                            psA1[:, g:g + 1],
                            wrec_w[:, (g * KC + k) * 128:(g * KC + k + 1) * 128],
                            red1u[:, k:k + 1] if bf16 else red1[:, k:k + 1],
                            start=(k == 0), stop=(k == KC - 1),
                        )
                a1sb = sp.tile([128, 4], F32, tag="a1sb")
                nc.vector.tensor_add(a1sb[:, :], psA1[:, :],
                                     xprojt_sb[:, 4 * (t + 1):4 * (t + 1) + 4])
                a1sb_prev = a1sb
            hnew_full_prev = red1
            h_new_prev = h_new

        # ===== epilogue: h_att(TR-1) =====
        t = TR
        parp = (t - 1) & 1
        red2 = sp.tile([128, 4], F32, tag="red2")
        if not no_comm:
            w2_inst = nc.vector.wait_ge(sem2, 14 * t)
            add_dep_helper(w2_inst.ins, lastfence[0].ins, False, "wait after fence")
        r2 = rec2[parp][:, :].rearrange("p (s c) -> p c s", s=N_CORES - 1)
        i3 = nc.vector.reduce_sum(red2[:, :], r2[:, :, :], axis=mybir.AxisListType.X)
        nc.vector.tensor_add(red2[:, :], red2[:, :], pay2[parp][:, 0:4])
        if not no_comm:
            add_dep_helper(i3.ins, w2_inst.ins, False, "gate recv2 on sem2")
        zpart = sp.tile([128, 1], F32, tag="zpart")
        nc.vector.reduce_sum(zpart[:, :], red2[:, :], axis=mybir.AxisListType.X)
        psZ = psp.tile([128, 1], F32, tag="psZ")
        nc.tensor.matmul(psZ[:, :], ones_sb[:, :], zpart[:, :], start=True, stop=True)
        rz = sp.tile([128, 1], F32, tag="rz")
        nc.vector.reciprocal(rz[:, :], psZ[:, :])
        if bf16:
            red2u = sp.tile([128, 4], wdt, tag="red2u")
            nc.vector.tensor_copy(red2u[:, :], red2[:, :])
        else:
            red2u = red2
        psNo = psp.tile([128, 1], F32, tag="psNo")
        for E in range(4):
            nc.tensor.matmul(
                psNo[:, :], hTj_w[:, 128 * E:128 * (E + 1)], red2u[:, E:E + 1],
                start=(E == 0), stop=(E == 3),
            )
        htmp = sp.tile([128, 1], F32, tag="htmp")
        nc.vector.tensor_add(htmp[:, :], h_new_prev[:, :], csumj_sb[:, :])
        nc.vector.scalar_tensor_tensor(
            houtbuf[:, t - 1:t], psNo[:, :], rz[:, :], htmp[:, :],
            mybir.AluOpType.mult, mybir.AluOpType.add)

        nc.sync.dma_start(hout.ap(), houtbuf[:, :])

    nc.compile()
    return nc


def prep_in_maps_v2(inputs: dict, T: int) -> list[dict]:
    maps = prep_in_maps(inputs, T)
    dup2 = np.concatenate([np.eye(RPC, dtype=np.float32)] * 2, axis=1)  # [64, 128]
    for j, m in enumerate(maps):
        emask = np.zeros((128, 4), np.float32)
        emask[(j % 2) * RPC:(j % 2 + 1) * RPC, j // 2] = 1.0
        m["emask"] = emask
        m["dup2"] = np.ascontiguousarray(dup2)
        m.pop("hrows")
        # v2 uses plain f32 for these
        W_iouh = np.asarray(inputs["W_iouh"], np.float32)
        W_fh = np.asarray(inputs["W_fh"], np.float32)
        W_attnh = np.asarray(inputs["W_attnh"], np.float32)
        H = np.asarray(inputs["hiddn_state_mat"], np.float32)
        # v2 gate order [i, o, f, u]
        gate_w = [W_iouh[:, 0:MEM], W_iouh[:, MEM:2 * MEM], W_fh, W_iouh[:, 2 * MEM:]]
        W_ioux = np.asarray(inputs["W_ioux"], np.float32)
        W_fx = np.asarray(inputs["W_fx"], np.float32)
        b_iou = (np.asarray(inputs["b_ioux"], np.float32)
                 + np.asarray(inputs["b_iouh"], np.float32))
        b_f = (np.asarray(inputs["b_fx"], np.float32)
               + np.asarray(inputs["b_fh"], np.float32))
        gate_wx = [W_ioux[:, 0:MEM], W_ioux[:, MEM:2 * MEM], W_fx, W_ioux[:, 2 * MEM:]]
        gate_b = [b_iou[0:MEM], b_iou[MEM:2 * MEM], b_f, b_iou[2 * MEM:]]
        wrec = np.zeros((128, 4 * KC * 128), np.float32)
        wx = np.zeros((128, 4 * KC * 128), np.float32)
        for g in range(4):
            for k in range(KC):
                sl = np.s_[:, (g * KC + k) * 128:(g * KC + k + 1) * 128]
                wrec[sl] = gate_w[g][128 * k:128 * (k + 1), 128 * j:128 * (j + 1)]
                wx[sl] = gate_wx[g][128 * k:128 * (k + 1), 128 * j:128 * (j + 1)]
        m["wrec"] = wrec
        m["wx"] = wx
        m["bias_x"] = np.ascontiguousarray(
            np.stack([gate_b[g][128 * j:128 * (j + 1)] for g in range(4)], axis=1))
        m["hTj"] = np.ascontiguousarray(np.concatenate(
            [H[128 * E:128 * (E + 1), 128 * j:128 * (j + 1)] for E in range(4)],
            axis=1))
        W1 = W_attnh[:MEM]
        w1t = np.zeros((128, KC * 128), np.float32)
        for mm in range(KC):
            w1t[:, mm * 128:(mm + 1) * 128] = \
                W1[128 * j:128 * (j + 1), 128 * mm:128 * (mm + 1)]
        m["w1t"] = w1t
    return maps


def postprocess_v2(houts: list, T: int) -> np.ndarray:
    # per-core hout [128, T]: core j owns mem dims 128j..128j+127
    buf = np.stack([np.asarray(h).reshape(128, T) for h in houts], axis=0)  # [8,128,T]
    return np.ascontiguousarray(buf.transpose(2, 0, 1).reshape(T, MEM)).astype(np.float32)


# ----------------------------------------------------------------------------
# Harness entry point: full (unsharded) inputs -> full output.
# ----------------------------------------------------------------------------
KERNEL_BF16 = True


def kernel(**inputs) -> np.ndarray:
    from concourse.bass_utils import run_bass_kernel_spmd

    T = int(np.asarray(inputs["inputs"]).shape[0])
    nc = build_nc_v2(T, bf16=KERNEL_BF16)
    in_maps = prep_in_maps_v2(inputs, T)
    res = run_bass_kernel_spmd(nc, in_maps, core_ids=list(range(N_CORES)))
    return postprocess_v2([res.results[c]["hout"] for c in range(N_CORES)], T)



# revision 34
# speedup vs baseline: 3.0431x; 3.0431x over previous
"""ChildSum TreeLSTM + attention, 8-core SPMD Trainium2 kernel.

SHIPPED CONFIG: kernel() -> build_nc_v3(hc=True) — single logical sync per
step realized as a 3-round XOR-hypercube allreduce (3 SWDGE preps + 3
triggers/step), attention fully replicated per core (all 512 rows, ACT-bias
tanh), bf16 stationary weights. Measured 37.5us/step = 9.59ms (vs 105us/step
= 26.9ms for the v2 baseline below) at l2 rel 6.7e-4.

Why hypercube: on real TRN2 a remote_dma_broadcast PREP instruction costs
~7us on the Pool engine and scales superlinearly with preps-per-step (7
preps ~46us), while a single prep+trigger+wait round trip is only ~4.8us.
v3's one-shot all-to-all (7 preps) was Pool-throughput-bound; 3 hypercube
rounds trade 2 extra ~2us latency hops for 4 fewer preps. A hub-reduce
topology (1 prep/core) would be better still but SWDGE preps inside
tc.If branches + explicit-count triggers hang the device (count=None bakes
the build-time pending count into the ISA; explicit counts fire before the
async Q7 desc commit) — see build_nc_v4 (NON-FUNCTIONAL, kept for notes).

v2 (below) is the original two-sync design; v3 replicates attention to
drop sync2; hc=True swaps sync1's implementation.
"""
"""v2 notes:

Sharding: mem_dim 1024 in 8 slices of 128 (core j owns slice j); the 512
attention rows in 8 groups of 64. Two XOR all-to-all syncs per step (7
single-dest remote_dma_broadcast preps each, self contribution added in the
receive reduction).

v2 critical-path design (vs the v1 baseline kept below for reference):
  - SWDGE descriptor preps are HOISTED off the critical path: sync1(t+1)
    preps enqueue right after trig2(t), sync2(t) preps after trig1(t); only
    trigger_dma (gated on a payload sem_inc fence) sits on the path.
  - Gate GEMV split: psA1 = Wr^T(csum + h_new_full) runs hidden under the
    attn+sync2 window; on the path only psA2 = (H Wr)^T e (16 MMs, K=e) and
    gates = ACT(psA1 - psA2/Z + xproj[t]).  g0 = Wr^T csum is folded into
    xproj (removed again at t=0 where h(-1)=0).
  - exp via sigmoid: e = sig(s) * recip(sig(-s)) keeps ACT on one table set
    (sigmoid+tanh) -> zero ~2.7us ACT table reloads per step.
  - attention tanh is ONE ACT [128,512]; the w bias is pre-added with a
    single DVE using a stride-0 broadcast AP.
  - numer moved post-sync2 and sharded: sync2 carries only the one-hot e
    columns [128,4]; each core computes its own numer slice (4 MMs) and its
    own h_att slice; host gathers the 8 per-core hout [128,T] outputs.
  - h_att/houtbuf writes are off-path; one final 128KB DMA.
  - ones matrix is -1 so 1/(-Z) fuses the subtractions into
    scalar_tensor_tensor ops.
  - bf16 stationary weights (wrec/hwr/w1t/hTj, FWL 2x weight load), fp32
    PSUM accumulate; rhs vectors cast per step. l2 rel vs fp32 ref: 7.8e-6.

Host prep does layout only (transpose/slice/cast) - zero FLOPs.
"""
import dataclasses
import numpy as np
from contextlib import ExitStack

import concourse.bass as bass
import concourse.tile as tile
from concourse import bacc, mybir
from concourse.bass import create_sync_update
from concourse.tile_rust import add_dep_helper

F32 = mybir.dt.float32
AF = mybir.ActivationFunctionType
N_CORES = 8
MEM = 1024
IN_DIM = 1024
MROWS = 512          # attention rows
KC = MEM // 128      # 8 column chunks
RPC = MROWS // N_CORES  # 64 attention rows per core

# Sems incremented by remote cores / DMA completion that the Tile scheduling
# pass (single-core sim, no_exec) can never see. Pre-satisfied there only;
# the runtime NEFF keeps the real waits.
_EXTERNAL_SEMS: list = []
_OrigCoreSim = tile.CoreSim


class _SchedCoreSim(_OrigCoreSim):
    def __init__(self, *a, **kw):
        super().__init__(*a, **kw)
        for sem in _EXTERNAL_SEMS:
            self.update_semaphore(create_sync_update(sem, 1 << 22))


tile.CoreSim = _SchedCoreSim


def prep_in_maps(inputs: dict, T: int, wdt_np=np.float32) -> list[dict]:
    """Host-side layout-only prep of per-core input maps."""
    X = np.asarray(inputs["inputs"], np.float32).reshape(T, IN_DIM)
    H = np.asarray(inputs["hiddn_state_mat"], np.float32)
    W_ioux = np.asarray(inputs["W_ioux"], np.float32)
    W_iouh = np.asarray(inputs["W_iouh"], np.float32)
    W_fx = np.asarray(inputs["W_fx"], np.float32)
    W_fh = np.asarray(inputs["W_fh"], np.float32)
    Wa = np.asarray(inputs["Wa"], np.float32).reshape(MEM)
    W_attnh = np.asarray(inputs["W_attnh"], np.float32)
    b_iou = (np.asarray(inputs["b_ioux"], np.float32)
             + np.asarray(inputs["b_iouh"], np.float32))
    b_f = (np.asarray(inputs["b_fx"], np.float32)
           + np.asarray(inputs["b_fh"], np.float32))
    b_attnh = np.asarray(inputs["b_attnh"], np.float32)

    W1 = W_attnh[:MEM]
    W2 = W_attnh[MEM:]

    # replicated tensors
    xT_l = X.T.reshape(KC, 128, T).transpose(1, 0, 2).reshape(128, KC * T)
    xT_l = np.ascontiguousarray(xT_l)
    # w2t tile(c,k) = W2[128k+a, 128c+b] -> [128, 64*128]
    w2t = np.zeros((128, KC * KC * 128), np.float32)
    for c in range(KC):
        for k in range(KC):
            w2t[:, (c * KC + k) * 128:(c * KC + k + 1) * 128] = \
                W2[128 * k:128 * (k + 1), 128 * c:128 * (c + 1)]
    hT_l = np.ascontiguousarray(
        H.T.reshape(KC, 128, MROWS).transpose(1, 0, 2).reshape(128, KC * MROWS))
    wa_l = np.ascontiguousarray(Wa.reshape(KC, 128).T)
    b2_l = np.ascontiguousarray(b_attnh.reshape(KC, 128).T)

    gate_w = [W_iouh[:, 0:MEM], W_iouh[:, MEM:2 * MEM], W_iouh[:, 2 * MEM:], W_fh]
    gate_wx = [W_ioux[:, 0:MEM], W_ioux[:, MEM:2 * MEM], W_ioux[:, 2 * MEM:], W_fx]
    gate_b = [b_iou[0:MEM], b_iou[MEM:2 * MEM], b_iou[2 * MEM:], b_f]

    maps = []
    for j in range(N_CORES):
        wrec = np.zeros((128, 4 * KC * 128), np.float32)
        wx = np.zeros((128, 4 * KC * 128), np.float32)
        for g in range(4):
            for k in range(KC):
                sl = np.s_[:, (g * KC + k) * 128:(g * KC + k + 1) * 128]
                wrec[sl] = gate_w[g][128 * k:128 * (k + 1), 128 * j:128 * (j + 1)]
                wx[sl] = gate_wx[g][128 * k:128 * (k + 1), 128 * j:128 * (j + 1)]
        w1t = np.zeros((128, KC * 128), np.float32)
        for m in range(KC):
            w1t[:, m * 128:(m + 1) * 128] = \
                W1[128 * j:128 * (j + 1), 128 * m:128 * (m + 1)]
        bias_x = np.stack([gate_b[g][128 * j:128 * (j + 1)] for g in range(4)], axis=1)
        mask = np.zeros((128, KC), np.float32)
        mask[:, j] = 1.0
        Hown = H[RPC * j:RPC * (j + 1)]            # [64, 1024]
        hTown_l = np.ascontiguousarray(
            Hown.T.reshape(KC, 128, RPC).transpose(1, 0, 2).reshape(128, KC * RPC))
        maps.append({
            "xT": xT_l, "wx": wx,
            "wrec": wrec.astype(wdt_np), "w1t": w1t.astype(wdt_np),
            "w2t": w2t, "hT": hT_l, "hTown": hTown_l,
            "hrows": np.ascontiguousarray(H[RPC * j:RPC * (j + 1)]).astype(wdt_np),
            "wa": wa_l, "bias_x": np.ascontiguousarray(bias_x),
            "bias2": b2_l, "mask": mask,
        })
    return maps


def postprocess(hout_core0: np.ndarray, T: int) -> np.ndarray:
    # hout [T, 128, KC]: [t, p, c] = h_att_t[128c + p]
    return np.ascontiguousarray(
        hout_core0.transpose(0, 2, 1).reshape(T, MEM)).astype(np.float32)




# ============================================================================
# v2: hoisted SWDGE preps (triggers only on the critical path), gate GEMV
# split A1 = Wr^T(csum+h_new) [hidden under attn+sync2] + A2 = (H Wr)^T e
# [16 MMs on path], exp via sigma(x)/sigma(-x) (no ACT table switches),
# h_att/output assembly off-path, hout batched into one final DMA.
# ============================================================================
def build_nc_v2(T: int, t_run: int | None = None, no_comm: bool = False,
                bf16: bool = False):
    del _EXTERNAL_SEMS[:]
    wdt = mybir.dt.bfloat16 if bf16 else F32
    nc = bacc.Bacc(dynamic_dma_scratch_size=32768)
    TR = t_run if t_run is not None else T

    dp = lambda n, s_, dt=F32: nc.declare_dram_parameter(n, s_, dt, isOutput=False)
    xT = dp("xT", [128, KC * T])            # xT[p, T*k+t] = X[t, 128k+p]
    wx = dp("wx", [128, 4 * KC * 128])      # tile(g,k): Wx_g[128k+a, 128j+b]
    wrec = dp("wrec", [128, 4 * KC * 128])  # tile(g,k): Wr_g[128k+a, 128j+b]
    w1t = dp("w1t", [128, KC * 128])        # tile m: W1[128j+a, 128m+b]
    w2t = dp("w2t", [128, KC * KC * 128])   # tile(c,k): W2[128k+a, 128c+b]
    hTs = dp("hT", [128, KC * MROWS])       # hT[p, 512m+i] = H[i, 128m+p]
    hTown = dp("hTown", [128, KC * RPC])    # hTown[p, 64k+i] = H[64j+i, 128k+p]
    hTj = dp("hTj", [128, MROWS])           # tile E: H[128E+i, 128j+p] (K=e rows)
    wa = dp("wa", [128, KC])                # wa[p,c] = Wa[128c+p]
    bias_x = dp("bias_x", [128, 4])         # per-gate bias for own slice
    bias2 = dp("bias2", [128, KC])          # b_attnh column form
    mask = dp("mask", [128, KC])            # one-hot col own_core
    emask = dp("emask", [128, 4])           # one-hot e-placement [rows (j%2)*64, col j//2]
    dup2 = dp("dup2", [RPC, 128])           # [I64 | I64]: dup e to both halves
    hout = nc.declare_dram_parameter("hout", [128, T], F32, isOutput=True)

    with tile.TileContext(nc) as tc, ExitStack() as ctx:
        paysem = ctx.enter_context(nc.semaphore("paysem"))
        sem1 = ctx.enter_context(nc.semaphore("rdma_sem1"))
        sem2 = ctx.enter_context(nc.semaphore("rdma_sem2"))
        lsem1 = ctx.enter_context(nc.semaphore("rdma_lsem1"))
        lsem2 = ctx.enter_context(nc.semaphore("rdma_lsem2"))
        _EXTERNAL_SEMS.extend([sem1, sem2, lsem1, lsem2])

        comm = ctx.enter_context(tc.tile_pool(name="comm", bufs=1))
        pay1 = [comm.tile([128, 16], F32, name=f"pay1_{p}", tag=f"pay1_{p}") for p in range(2)]
        rec1 = [comm.tile([128, 112], F32, name=f"rec1_{p}", tag=f"rec1_{p}") for p in range(2)]
        pay2 = [comm.tile([128, 4], F32, name=f"pay2_{p}", tag=f"pay2_{p}") for p in range(2)]
        rec2 = [comm.tile([128, 28], F32, name=f"rec2_{p}", tag=f"rec2_{p}") for p in range(2)]

        const = ctx.enter_context(tc.tile_pool(name="const", bufs=1))
        wrec_sb = const.tile([128, 4 * KC * 128], F32, tag="wrec")
        w1t_sb = const.tile([128, KC * 128], F32, tag="w1t")
        hTj_sb = const.tile([128, MROWS], F32, tag="hTj")
        csumj_sb = const.tile([128, 1], F32, tag="csumj")
        wa_sb = const.tile([128, KC], F32, tag="wa")
        hw2T_sb = const.tile([128, KC * RPC], F32, tag="hw2T")
        hwr_sb = const.tile([128, 16 * 128], wdt, tag="hwr")   # tile(g,E): HWr[128E+e, g-col]
        xproj_sb = const.tile([128, 4 * T], F32, tag="xproj")
        xprojt_sb = const.tile([128, 4 * T], F32, tag="xprojt")
        csum_sb = const.tile([128, KC], F32, tag="csum")
        ones_sb = const.tile([128, 128], F32, tag="ones")
        mask_sb = const.tile([128, KC], F32, tag="mask")
        emask_sb = const.tile([128, 4], F32, tag="emask")
        dup2_sb = const.tile([RPC, 128], F32, tag="dup2")
        zero4_sb = const.tile([128, 4], F32, tag="zero4")
        houtbuf = const.tile([128, T], F32, tag="houtbuf")
        if bf16:
            wrec_w = const.tile([128, 4 * KC * 128], wdt, tag="wrec_w")
            hTj_w = const.tile([128, MROWS], wdt, tag="hTj_w")

        nc.sync.dma_start(wrec_sb[:, :], wrec.ap())
        nc.sync.dma_start(w1t_sb[:, :], w1t.ap())
        nc.sync.dma_start(hTj_sb[:, :], hTj.ap())
        nc.sync.dma_start(wa_sb[:, :], wa.ap())
        nc.sync.dma_start(mask_sb[:, :], mask.ap())
        nc.sync.dma_start(emask_sb[:, :], emask.ap())
        nc.sync.dma_start(dup2_sb[:, :], dup2.ap())
        nc.vector.memset(ones_sb[:, :], -1.0)  # psZ = -Z so rz = -1/Z (fused adds)
        if bf16:
            nc.vector.tensor_copy(wrec_w[:, :], wrec_sb[:, :])
            nc.vector.tensor_copy(hTj_w[:, :], hTj_sb[:, :])
        else:
            wrec_w, hTj_w = wrec_sb, hTj_sb
        nc.vector.memset(zero4_sb[:, :], 0.0)
        nc.vector.memset(houtbuf[:, :], 0.0)
        for p in range(2):
            nc.vector.memset(pay2[p][:, :], 0.0)
        if no_comm:
            for p in range(2):
                nc.vector.memset(rec1[p][:, :], 0.01)
                nc.vector.memset(rec2[p][:, :], 0.01)

        # ---------- device precompute ----------
        with tc.tile_pool(name="pre", bufs=1) as pre, \
             tc.tile_pool(name="prepsum", bufs=1, space="PSUM") as pps:
            xT_sb = pre.tile([128, KC * T], F32, tag="xT")
            wx_sb = pre.tile([128, 4 * KC * 128], F32, tag="wx")
            w2t_sb = pre.tile([128, KC * KC * 128], F32, tag="w2t")
            hT_sb = pre.tile([128, KC * MROWS], F32, tag="hT")
            hTown_sb = pre.tile([128, KC * RPC], F32, tag="hTown")
            bx_sb = pre.tile([128, 4], F32, tag="bias_x")
            b2_sb = pre.tile([128, KC], F32, tag="bias2")
            nc.sync.dma_start(xT_sb[:, :], xT.ap())
            nc.sync.dma_start(wx_sb[:, :], wx.ap())
            nc.sync.dma_start(w2t_sb[:, :], w2t.ap())
            nc.sync.dma_start(hT_sb[:, :], hTs.ap())
            nc.sync.dma_start(hTown_sb[:, :], hTown.ap())
            nc.sync.dma_start(bx_sb[:, :], bias_x.ap())
            nc.sync.dma_start(b2_sb[:, :], bias2.ap())

            # colsumH column form
            for m in range(KC):
                nc.vector.reduce_sum(
                    csum_sb[:, m:m + 1],
                    hT_sb[:, m * MROWS:(m + 1) * MROWS],
                    axis=mybir.AxisListType.X,
                )

            # own column of csum (one-hot mask over the 8 chunk cols)
            cm = pre.tile([128, KC], F32, tag="cmask")
            nc.vector.tensor_mul(cm[:, :], csum_sb[:, :], mask_sb[:, :])
            nc.vector.reduce_sum(csumj_sb[:, :], cm[:, :], axis=mybir.AxisListType.X)

            # g0 = Wr^T csum (own 4 gate cols); combined bias = bias_x + g0
            psg0 = pps.tile([128, 4], F32, tag="ps_g0")
            for g in range(4):
                for k in range(KC):
                    nc.tensor.matmul(
                        psg0[:, g:g + 1],
                        wrec_sb[:, (g * KC + k) * 128:(g * KC + k + 1) * 128],
                        csum_sb[:, k:k + 1],
                        start=(k == 0), stop=(k == KC - 1),
                    )
            biasc = pre.tile([128, 4], F32, tag="biasc")
            nc.vector.tensor_add(biasc[:, :], psg0[:, :], bx_sb[:, :])

            # xproj[g]: [128, T] = sum_k Wx_g[k]^T @ xT[k] (+ combined bias)
            for g in range(4):
                ps = pps.tile([128, T], F32, tag="ps_x")
                for k in range(KC):
                    nc.tensor.matmul(
                        ps[:, :],
                        wx_sb[:, (g * KC + k) * 128:(g * KC + k + 1) * 128],
                        xT_sb[:, k * T:(k + 1) * T],
                        start=(k == 0), stop=(k == KC - 1),
                    )
                nc.vector.tensor_scalar_add(
                    xproj_sb[:, g * T:(g + 1) * T], ps[:, :], biasc[:, g:g + 1]
                )
                # h(-1) = 0: no csum term at t=0, so remove the folded g0
                nc.vector.tensor_sub(
                    xproj_sb[:, g * T:g * T + 1],
                    xproj_sb[:, g * T:g * T + 1], psg0[:, g:g + 1]
                )
            # repack g-major -> t-major: xprojt[:, 4t+g] = xproj[:, g*T+t]
            nc.vector.tensor_copy(
                xprojt_sb[:, :].rearrange("p (t g) -> p t g", g=4),
                xproj_sb[:, :].rearrange("p (g t) -> p t g", g=4),
            )

            # hw2T c-chunk: [128, 64] = sum_k W2[c,k]^T @ hTown[k]  (+ b2)
            for c in range(KC):
                ps2 = pps.tile([128, RPC], F32, tag="ps_h")
                for k in range(KC):
                    nc.tensor.matmul(
                        ps2[:, :],
                        w2t_sb[:, (c * KC + k) * 128:(c * KC + k + 1) * 128],
                        hTown_sb[:, k * RPC:(k + 1) * RPC],
                        start=(k == 0), stop=(k == KC - 1),
                    )
                nc.vector.tensor_scalar_add(
                    hw2T_sb[:, c * RPC:(c + 1) * RPC], ps2[:, :], b2_sb[:, c:c + 1]
                )

            # hwr tile(g,E) = HWr[128E:128E+128, own gate col g] = sum_k H[E,k] Wr[k,g]
            for g in range(4):
                for E in range(4):
                    ps3 = pps.tile([128, 128], F32, tag="ps_hwr")
                    for k in range(KC):
                        nc.tensor.matmul(
                            ps3[:, :],
                            hT_sb[:, k * MROWS + 128 * E:k * MROWS + 128 * (E + 1)],
                            wrec_sb[:, (g * KC + k) * 128:(g * KC + k + 1) * 128],
                            start=(k == 0), stop=(k == KC - 1),
                        )
                    nc.vector.tensor_copy(
                        hwr_sb[:, (g * 4 + E) * 128:(g * 4 + E + 1) * 128], ps3[:, :]
                    )

        # ---------- state & per-step pools ----------
        sp = ctx.enter_context(tc.tile_pool(name="step", bufs=2))
        psp = ctx.enter_context(tc.tile_pool(name="spsum", bufs=1, space="PSUM"))

        chain = [None]
        nprep = [0]

        def preps(pay, recs, rsem, lsem):
            prev = chain[0]
            for k in range(1, N_CORES):
                rdests = [None] * N_CORES
                rdests[k] = (0, k)
                inst = nc.gpsimd.remote_dma_broadcast(
                    out_ap=recs(k), in_ap=pay, remote_sem=rsem, local_sem=lsem,
                    rdests=rdests,
                )
                if prev is not None:
                    add_dep_helper(inst.ins, prev, False, "swdge ring order")
                prev = inst.ins
            chain[0] = prev
            nprep[0] += 1

        npay = [0]

        lastfence = [None]

        def trig(payload_insts):
            # count=None: Tile manages prep->trigger HW ordering. Payload gate:
            # the DMA reads the pay tile only at fire time, and the preps were
            # emitted before this step's payload writes. DVE is in-order, so an
            # explicit sem_inc ordered after the payload writes proves them
            # complete; the trigger waits on paysem. A dedicated sem avoids the
            # engine-clock granularity deadlock (trigger must not wait on DVE
            # ticks past the sem1/sem2 waits).
            fence = nc.vector.sem_inc(paysem, 1)
            for pi in payload_insts:
                add_dep_helper(fence.ins, pi.ins, False, "fence after payload")
            if lastfence[0] is not None:
                add_dep_helper(fence.ins, lastfence[0].ins, False, "fence order")
            lastfence[0] = fence
            npay[0] += 1
            w = nc.gpsimd.wait_ge(paysem, npay[0])
            add_dep_helper(w.ins, chain[0], False, "swdge ring order")
            tg = nc.gpsimd.trigger_dma(count=None)
            add_dep_helper(tg.ins, w.ins, False, "swdge ring order")
            chain[0] = tg.ins

        gates3 = comm.tile([128, 1], F32, tag="gates3")
        ccol = sp.tile([128, 1], F32, tag="ccol")
        nc.vector.memset(ccol[:, :], 0.0)
        hnew_full_prev = None
        h_new_prev = None
        a1sb_prev = None
        psA1 = psp.tile([128, 4], F32, tag="psA1")

        for t in range(TR):
            par = t & 1
            parp = (t - 1) & 1

            if t == 0 and not no_comm:
                preps(pay1[0][:, 0:16],
                      lambda k: rec1[0][:, (k - 1) * 16:k * 16], sem1, lsem1)

            # ===== post-sync2 of step t-1: rz, A2, h_att(t-1) =====
            if t >= 1:
                red2 = sp.tile([128, 4], F32, tag="red2")
                if not no_comm:
                    w2_inst = nc.vector.wait_ge(sem2, 14 * t)
                    add_dep_helper(w2_inst.ins, lastfence[0].ins, False, "wait after fence")
                r2 = rec2[parp][:, :].rearrange("p (s c) -> p c s", s=N_CORES - 1)
                i3 = nc.vector.reduce_sum(red2[:, :], r2[:, :, :], axis=mybir.AxisListType.X)
                nc.vector.tensor_add(red2[:, :], red2[:, :], pay2[parp][:, 0:4])
                if not no_comm:
                    add_dep_helper(i3.ins, w2_inst.ins, False, "gate recv2 on sem2")
                # Z and 1/Z first (psZ ahead of A2 in the PE FIFO)
                zpart = sp.tile([128, 1], F32, tag="zpart")
                nc.vector.reduce_sum(zpart[:, :], red2[:, :], axis=mybir.AxisListType.X)
                psZ = psp.tile([128, 1], F32, tag="psZ")
                nc.tensor.matmul(psZ[:, :], ones_sb[:, :], zpart[:, :], start=True, stop=True)
                rz = sp.tile([128, 1], F32, tag="rz")
                nc.vector.reciprocal(rz[:, :], psZ[:, :])
                # A2 = hwr^T @ ecol (on path)
                if bf16:
                    red2u = sp.tile([128, 4], wdt, tag="red2u")
                    nc.vector.tensor_copy(red2u[:, :], red2[:, :])
                else:
                    red2u = red2
                psA2 = psp.tile([128, 4], F32, tag="psA2")
                for g in (3, 0, 1, 2):
                    for E in range(4):
                        nc.tensor.matmul(
                            psA2[:, g:g + 1],
                            hwr_sb[:, (g * 4 + E) * 128:(g * 4 + E + 1) * 128],
                            red2u[:, E:E + 1],
                            start=(E == 0), stop=(E == 3),
                        )
                    if g == 3:
                        dpre3 = sp.tile([128, 1], F32, tag="dpre3")
                        nc.vector.scalar_tensor_tensor(
                            dpre3[:, :], psA2[:, 3:4], rz[:, :], a1sb_prev[:, 3:4],
                            mybir.AluOpType.mult, mybir.AluOpType.add)
                        nc.scalar.activation(gates3[:, :], dpre3[:, :], AF.Tanh)
                # own slice of numer + h_att(t-1) output (off critical path)
                psNo = psp.tile([128, 1], F32, tag="psNo")
                for E in range(4):
                    nc.tensor.matmul(
                        psNo[:, :], hTj_w[:, 128 * E:128 * (E + 1)], red2u[:, E:E + 1],
                        start=(E == 0), stop=(E == 3),
                    )
                htmp = sp.tile([128, 1], F32, tag="htmp")
                nc.vector.tensor_add(htmp[:, :], h_new_prev[:, :], csumj_sb[:, :])
                nc.vector.scalar_tensor_tensor(
                    houtbuf[:, t - 1:t], psNo[:, :], rz[:, :], htmp[:, :],
                    mybir.AluOpType.mult, mybir.AluOpType.add)

            # ===== gates(t) =====
            # gate order [i, o, f, u]: sigmoid on 0:3; tanh(u) done early (gates3)
            gates = sp.tile([128, 4], F32, tag="gates")
            if t == 0:
                nc.scalar.activation(gates[:, 0:3], xprojt_sb[:, 4 * t:4 * t + 3], AF.Sigmoid)
                nc.scalar.activation(gates3[:, :], xprojt_sb[:, 4 * t + 3:4 * t + 4], AF.Tanh)
            else:
                dpre = sp.tile([128, 3], F32, tag="dpre")
                nc.vector.scalar_tensor_tensor(
                    dpre[:, :], psA2[:, 0:3], rz[:, :], a1sb_prev[:, 0:3],
                    mybir.AluOpType.mult, mybir.AluOpType.add)
                nc.scalar.activation(gates[:, 0:3], dpre[:, :], AF.Sigmoid)
            iu = sp.tile([128, 1], F32, tag="iu")
            nc.vector.tensor_mul(iu[:, :], gates[:, 0:1], gates3[:, :])
            ccol_new = sp.tile([128, 1], F32, tag="ccol")
            nc.vector.scalar_tensor_tensor(
                ccol_new[:, :], ccol[:, :], gates[:, 2:3], iu[:, :],
                mybir.AluOpType.mult, mybir.AluOpType.add)
            ccol = ccol_new
            tanh_c = sp.tile([128, 1], F32, tag="tanh_c")
            nc.scalar.activation(tanh_c[:, :], ccol[:, :], AF.Tanh)
            h_new = sp.tile([128, 1], F32, tag="h_new")
            hn_inst = nc.vector.tensor_mul(h_new[:, :], gates[:, 1:2], tanh_c[:, :])

            # ===== pay1 = [h_new one-hot | W1^T h_new] ; TRIG1 =====
            psW = psp.tile([128, KC], F32, tag="psW")
            for m in range(KC):
                nc.tensor.matmul(
                    psW[:, m:m + 1], w1t_sb[:, m * 128:(m + 1) * 128], h_new[:, :],
                    start=True, stop=True,
                )
            if t >= 2 and not no_comm:
                lw1 = nc.vector.wait_ge(lsem1, 112 * t)
                add_dep_helper(lw1.ins, hn_inst.ins, False, "anchor lsem1 wait")
                add_dep_helper(lw1.ins, lastfence[0].ins, False, "wait after fence")
            mm1 = nc.vector.tensor_scalar_mul(pay1[par][:, 0:8], mask_sb[:, :], h_new[:, :])
            cp1 = nc.vector.tensor_copy(pay1[par][:, 8:16], psW[:, :])
            if t >= 2 and not no_comm:
                add_dep_helper(mm1.ins, lw1.ins, False, "pay1 WAR")
                add_dep_helper(cp1.ins, lw1.ins, False, "pay1 WAR")
            if not no_comm:
                trig([mm1, cp1])
                # enqueue sync2(t) preps now (off-path, overlaps flight1+attn)
                preps(pay2[par][:, 0:4],
                      lambda k: rec2[par][:, (k - 1) * 4:k * 4], sem2, lsem2)
                w1_inst = nc.vector.wait_ge(sem1, 14 * (t + 1))
                add_dep_helper(w1_inst.ins, cp1.ins, False, "anchor sem1 wait")
                add_dep_helper(w1_inst.ins, lastfence[0].ins, False, "wait after fence")

            red1 = sp.tile([128, 16], F32, tag="red1")
            r1 = rec1[par][:, :].rearrange("p (s c) -> p c s", s=N_CORES - 1)
            # w_sum half first: it is the only on-path consumer (attention
            # bias); the h_new half feeds only the off-path psA1 GEMV.
            i1 = nc.vector.reduce_sum(red1[:, 8:16], r1[:, 8:16, :], axis=mybir.AxisListType.X)
            nc.vector.tensor_add(red1[:, 8:16], red1[:, 8:16], pay1[par][:, 8:16])
            ih = nc.vector.reduce_sum(red1[:, 0:8], r1[:, 0:8, :], axis=mybir.AxisListType.X)
            nc.vector.tensor_add(red1[:, 0:8], red1[:, 0:8], pay1[par][:, 0:8])
            if not no_comm:
                add_dep_helper(i1.ins, w1_inst.ins, False, "gate recv1 on sem1")
                add_dep_helper(ih.ins, w1_inst.ins, False, "gate recv1 on sem1")

            # ===== attention =====
            tT = sp.tile([128, KC * RPC], F32, tag="tT")
            tpre = sp.tile([128, KC * RPC], F32, tag="tpre")
            wv = red1[:, 8:16]
            wb = dataclasses.replace(wv, ap=[*wv.ap, [0, RPC]])
            nc.vector.tensor_tensor(
                tpre[:, :].rearrange("p (c i) -> p c i", c=KC),
                hw2T_sb[:, :].rearrange("p (c i) -> p c i", c=KC),
                wb, mybir.AluOpType.add)
            half = KC * RPC // 2
            nc.scalar.activation(tT[:, 0:half], tpre[:, 0:half], AF.Tanh)
            nc.scalar.activation(tT[:, half:], tpre[:, half:], AF.Tanh)
            psS = psp.tile([RPC, 1], F32, tag="psS")
            for c in range(KC):
                nc.tensor.matmul(
                    psS[:, :], tT[:, c * RPC:(c + 1) * RPC], wa_sb[:, c:c + 1],
                    start=(c == 0), stop=(c == KC - 1),
                )
            ep = sp.tile([RPC, 1], F32, tag="ep")
            nc.scalar.activation(ep[:, :], psS[:, :], AF.Sigmoid)
            en = sp.tile([RPC, 1], F32, tag="en")
            nc.scalar.activation(en[:, :], psS[:, :], AF.Sigmoid, scale=-1.0)
            ren = sp.tile([RPC, 1], F32, tag="ren")
            nc.vector.reciprocal(ren[:, :], en[:, :])
            e_own = sp.tile([RPC, 1], F32, tag="e_own")
            ei = nc.vector.tensor_mul(e_own[:, :], ep[:, :], ren[:, :])

            psE = psp.tile([128, 1], F32, tag="psE")
            nc.tensor.matmul(psE[:, :], dup2_sb[:, :], e_own[:, :], start=True, stop=True)

            if t >= 2 and not no_comm:
                lw2 = nc.vector.wait_ge(lsem2, 112 * t)
                add_dep_helper(lw2.ins, ei.ins, False, "anchor lsem2 wait")
                add_dep_helper(lw2.ins, lastfence[0].ins, False, "wait after fence")
            em2 = nc.vector.tensor_scalar_mul(pay2[par][:, 0:4], emask_sb[:, :], psE[:, :])
            if t >= 2 and not no_comm:
                add_dep_helper(em2.ins, lw2.ins, False, "pay2 WAR")
            if not no_comm:
                trig([em2])
                if t + 1 < TR:
                    # enqueue sync1(t+1) preps (off-path, overlaps flight2)
                    preps(pay1[1 - par][:, 0:16],
                          lambda k: rec1[1 - par][:, (k - 1) * 16:k * 16], sem1, lsem1)

            # ===== psA1(t+1) = Wr^T hnew_full  (hidden under attn+sync2) =====
            if t + 1 < TR:
                if bf16:
                    red1u = sp.tile([128, KC], wdt, tag="red1u")
                    nc.vector.tensor_copy(red1u[:, :], red1[:, 0:8])
                else:
                    red1u = None
                for g in range(4):
                    for k in range(KC):
                        nc.tensor.matmul(
                            psA1[:, g:g + 1],
                            wrec_w[:, (g * KC + k) * 128:(g * KC + k + 1) * 128],
                            red1u[:, k:k + 1] if bf16 else red1[:, k:k + 1],
                            start=(k == 0), stop=(k == KC - 1),
                        )
                a1sb = sp.tile([128, 4], F32, tag="a1sb")
                nc.vector.tensor_add(a1sb[:, :], psA1[:, :],
                                     xprojt_sb[:, 4 * (t + 1):4 * (t + 1) + 4])
                a1sb_prev = a1sb
            hnew_full_prev = red1
            h_new_prev = h_new

        # ===== epilogue: h_att(TR-1) =====
        t = TR
        parp = (t - 1) & 1
        red2 = sp.tile([128, 4], F32, tag="red2")
        if not no_comm:
            w2_inst = nc.vector.wait_ge(sem2, 14 * t)
            add_dep_helper(w2_inst.ins, lastfence[0].ins, False, "wait after fence")
        r2 = rec2[parp][:, :].rearrange("p (s c) -> p c s", s=N_CORES - 1)
        i3 = nc.vector.reduce_sum(red2[:, :], r2[:, :, :], axis=mybir.AxisListType.X)
        nc.vector.tensor_add(red2[:, :], red2[:, :], pay2[parp][:, 0:4])
        if not no_comm:
            add_dep_helper(i3.ins, w2_inst.ins, False, "gate recv2 on sem2")
        zpart = sp.tile([128, 1], F32, tag="zpart")
        nc.vector.reduce_sum(zpart[:, :], red2[:, :], axis=mybir.AxisListType.X)
        psZ = psp.tile([128, 1], F32, tag="psZ")
        nc.tensor.matmul(psZ[:, :], ones_sb[:, :], zpart[:, :], start=True, stop=True)
        rz = sp.tile([128, 1], F32, tag="rz")
        nc.vector.reciprocal(rz[:, :], psZ[:, :])
        if bf16:
            red2u = sp.tile([128, 4], wdt, tag="red2u")
            nc.vector.tensor_copy(red2u[:, :], red2[:, :])
        else:
            red2u = red2
        psNo = psp.tile([128, 1], F32, tag="psNo")
        for E in range(4):
            nc.tensor.matmul(
                psNo[:, :], hTj_w[:, 128 * E:128 * (E + 1)], red2u[:, E:E + 1],
                start=(E == 0), stop=(E == 3),
            )
        htmp = sp.tile([128, 1], F32, tag="htmp")
        nc.vector.tensor_add(htmp[:, :], h_new_prev[:, :], csumj_sb[:, :])
        nc.vector.scalar_tensor_tensor(
            houtbuf[:, t - 1:t], psNo[:, :], rz[:, :], htmp[:, :],
            mybir.AluOpType.mult, mybir.AluOpType.add)

        nc.sync.dma_start(hout.ap(), houtbuf[:, :])

    nc.compile()
    return nc


# ============================================================================
# v3: ONE sync per step. Attention is fully replicated (all 512 rows on every
# core): after sync1 delivers h_new_full + w = W1^T h_new, each core computes
# tanh/scores/e/Z locally (identical on all cores), so sync2 disappears —
# 7 SWDGE preps + 1 trigger + 1 global latency barrier per step removed.
# Per-core work per step: 8 ACT tanh [128,512] (bias fused via ACT bias=w_c),
# 32 psS MMs, e=sig(s)/sig(-s), psA2 16 MMs, gates, psW 8 MMs, 1 trigger.
# h_att output is written in-iteration (no epilogue).
# ============================================================================
def build_nc_v3(T: int, t_run: int | None = None, no_comm: bool = False,
                bf16: bool = False, free_run: bool = False, hc: bool = False):
    """hc=True: 3-round XOR-hypercube allreduce (3 preps+3 triggers/step,
    rounds XOR 1/2/4 with partial-sum combine) instead of the 7-prep
    single-shot all-to-all. Symmetric, no If branches."""
    del _EXTERNAL_SEMS[:]
    wdt = mybir.dt.bfloat16 if bf16 else F32
    nc = bacc.Bacc(dynamic_dma_scratch_size=32768)
    TR = t_run if t_run is not None else T

    dp = lambda n, s_, dt=F32: nc.declare_dram_parameter(n, s_, dt, isOutput=False)
    xT = dp("xT", [128, KC * T])            # xT[p, T*k+t] = X[t, 128k+p]
    wx = dp("wx", [128, 4 * KC * 128])      # tile(g,k): Wx_g[128k+a, 128j+b]
    wrec = dp("wrec", [128, 4 * KC * 128])  # tile(g,k): Wr_g[128k+a, 128j+b]
    w1t = dp("w1t", [128, KC * 128])        # tile m: W1[128j+a, 128m+b]
    w2t = dp("w2t", [128, KC * KC * 128])   # tile(c,k): W2[128k+a, 128c+b]
    hTs = dp("hT", [128, KC * MROWS])       # hT[p, 512m+i] = H[i, 128m+p]
    hTj = dp("hTj", [128, MROWS])           # tile E: H[128E+i, 128j+p]
    wa = dp("wa", [128, KC])                # wa[p,c] = Wa[128c+p]
    bias_x = dp("bias_x", [128, 4])         # per-gate bias for own slice
    bias2 = dp("bias2", [128, KC])          # b_attnh column form
    mask = dp("mask", [128, KC])            # one-hot col own_core
    hout = nc.declare_dram_parameter("hout", [128, T], F32, isOutput=True)

    with tile.TileContext(nc) as tc, ExitStack() as ctx:
        paysem = ctx.enter_context(nc.semaphore("paysem"))
        sem1 = ctx.enter_context(nc.semaphore("rdma_sem1"))
        lsem1 = ctx.enter_context(nc.semaphore("rdma_lsem1"))
        _EXTERNAL_SEMS.extend([sem1, lsem1])
        if hc:
            semr = [ctx.enter_context(nc.semaphore(f"rdma_semr{r}")) for r in range(3)]
            lsemr = [ctx.enter_context(nc.semaphore(f"rdma_lsemr{r}")) for r in range(3)]
            fsem = [ctx.enter_context(nc.semaphore(f"fsem{r}")) for r in range(3)]
            _EXTERNAL_SEMS.extend(semr + lsemr + fsem)

        comm = ctx.enter_context(tc.tile_pool(name="comm", bufs=1))
        pay1 = [comm.tile([128, 16], F32, name=f"pay1_{p}", tag=f"pay1_{p}") for p in range(2)]
        rec1 = [comm.tile([128, 112], F32, name=f"rec1_{p}", tag=f"rec1_{p}") for p in range(2)]
        if hc:
            recr = [[comm.tile([128, 16], F32, name=f"recr{r}_{p}", tag=f"recr{r}_{p}")
                     for p in range(2)] for r in range(3)]
            sbuf = [[comm.tile([128, 16], F32, name=f"s{r}_{p}", tag=f"s{r}_{p}")
                     for p in range(2)] for r in range(2)]  # s1, s2 partials

        const = ctx.enter_context(tc.tile_pool(name="const", bufs=1))
        wrec_sb = const.tile([128, 4 * KC * 128], F32, tag="wrec")
        w1t_sb = const.tile([128, KC * 128], F32, tag="w1t")
        hTj_sb = const.tile([128, MROWS], F32, tag="hTj")
        csumj_sb = const.tile([128, 1], F32, tag="csumj")
        wa_sb = const.tile([128, KC], F32, tag="wa")
        hw2T_sb = const.tile([128, KC * MROWS], F32, tag="hw2T")  # ALL 512 rows
        hwr_sb = const.tile([128, 16 * 128], wdt, tag="hwr")
        xproj_sb = const.tile([128, 4 * T], F32, tag="xproj")
        xprojt_sb = const.tile([128, 4 * T], F32, tag="xprojt")
        csum_sb = const.tile([128, KC], F32, tag="csum")
        ones_sb = const.tile([128, 128], F32, tag="ones")
        mask_sb = const.tile([128, KC], F32, tag="mask")
        houtbuf = const.tile([128, T], F32, tag="houtbuf")
        if bf16:
            wrec_w = const.tile([128, 4 * KC * 128], wdt, tag="wrec_w")
            hTj_w = const.tile([128, MROWS], wdt, tag="hTj_w")

        nc.sync.dma_start(wrec_sb[:, :], wrec.ap())
        nc.sync.dma_start(w1t_sb[:, :], w1t.ap())
        nc.sync.dma_start(hTj_sb[:, :], hTj.ap())
        nc.sync.dma_start(wa_sb[:, :], wa.ap())
        nc.sync.dma_start(mask_sb[:, :], mask.ap())
        nc.vector.memset(ones_sb[:, :], -1.0)  # psZ = -Z so rz = -1/Z
        if bf16:
            nc.vector.tensor_copy(wrec_w[:, :], wrec_sb[:, :])
            nc.vector.tensor_copy(hTj_w[:, :], hTj_sb[:, :])
        else:
            wrec_w, hTj_w = wrec_sb, hTj_sb
        nc.vector.memset(houtbuf[:, :], 0.0)
        if no_comm:
            for p in range(2):
                nc.vector.memset(rec1[p][:, :], 0.01)
                nc.vector.memset(pay1[p][:, :], 0.01)

        # ---------- device precompute ----------
        with tc.tile_pool(name="pre", bufs=1) as pre, \
             tc.tile_pool(name="prepsum", bufs=1, space="PSUM") as pps:
            xT_sb = pre.tile([128, KC * T], F32, tag="xT")
            wx_sb = pre.tile([128, 4 * KC * 128], F32, tag="wx")
            w2t_sb = pre.tile([128, KC * KC * 128], F32, tag="w2t")
            hT_sb = pre.tile([128, KC * MROWS], F32, tag="hT")
            bx_sb = pre.tile([128, 4], F32, tag="bias_x")
            b2_sb = pre.tile([128, KC], F32, tag="bias2")
            nc.sync.dma_start(xT_sb[:, :], xT.ap())
            nc.sync.dma_start(wx_sb[:, :], wx.ap())
            nc.sync.dma_start(w2t_sb[:, :], w2t.ap())
            nc.sync.dma_start(hT_sb[:, :], hTs.ap())
            nc.sync.dma_start(bx_sb[:, :], bias_x.ap())
            nc.sync.dma_start(b2_sb[:, :], bias2.ap())

            # colsumH column form
            for m in range(KC):
                nc.vector.reduce_sum(
                    csum_sb[:, m:m + 1],
                    hT_sb[:, m * MROWS:(m + 1) * MROWS],
                    axis=mybir.AxisListType.X,
                )

            cm = pre.tile([128, KC], F32, tag="cmask")
            nc.vector.tensor_mul(cm[:, :], csum_sb[:, :], mask_sb[:, :])
            nc.vector.reduce_sum(csumj_sb[:, :], cm[:, :], axis=mybir.AxisListType.X)

            # g0 = Wr^T csum (own 4 gate cols); combined bias = bias_x + g0
            psg0 = pps.tile([128, 4], F32, tag="ps_g0")
            for g in range(4):
                for k in range(KC):
                    nc.tensor.matmul(
                        psg0[:, g:g + 1],
                        wrec_sb[:, (g * KC + k) * 128:(g * KC + k + 1) * 128],
                        csum_sb[:, k:k + 1],
                        start=(k == 0), stop=(k == KC - 1),
                    )
            biasc = pre.tile([128, 4], F32, tag="biasc")
            nc.vector.tensor_add(biasc[:, :], psg0[:, :], bx_sb[:, :])

            # xproj[g]: [128, T] = sum_k Wx_g[k]^T @ xT[k] (+ combined bias)
            for g in range(4):
                ps = pps.tile([128, T], F32, tag="ps_x")
                for k in range(KC):
                    nc.tensor.matmul(
                        ps[:, :],
                        wx_sb[:, (g * KC + k) * 128:(g * KC + k + 1) * 128],
                        xT_sb[:, k * T:(k + 1) * T],
                        start=(k == 0), stop=(k == KC - 1),
                    )
                nc.vector.tensor_scalar_add(
                    xproj_sb[:, g * T:(g + 1) * T], ps[:, :], biasc[:, g:g + 1]
                )
                nc.vector.tensor_sub(
                    xproj_sb[:, g * T:g * T + 1],
                    xproj_sb[:, g * T:g * T + 1], psg0[:, g:g + 1]
                )
            nc.vector.tensor_copy(
                xprojt_sb[:, :].rearrange("p (t g) -> p t g", g=4),
                xproj_sb[:, :].rearrange("p (g t) -> p t g", g=4),
            )

            # hw2T full: chunk c [128, 512] = sum_k W2[c,k]^T @ hT[k] (+ b2)
            for c in range(KC):
                ps2 = pps.tile([128, MROWS], F32, tag="ps_h")
                for k in range(KC):
                    nc.tensor.matmul(
                        ps2[:, :],
                        w2t_sb[:, (c * KC + k) * 128:(c * KC + k + 1) * 128],
                        hT_sb[:, k * MROWS:(k + 1) * MROWS],
                        start=(k == 0), stop=(k == KC - 1),
                    )
                nc.vector.tensor_scalar_add(
                    hw2T_sb[:, c * MROWS:(c + 1) * MROWS], ps2[:, :], b2_sb[:, c:c + 1]
                )

            # hwr tile(g,E) = HWr[128E:128E+128, own gate col g]
            for g in range(4):
                for E in range(4):
                    ps3 = pps.tile([128, 128], F32, tag="ps_hwr")
                    for k in range(KC):
                        nc.tensor.matmul(
                            ps3[:, :],
                            hT_sb[:, k * MROWS + 128 * E:k * MROWS + 128 * (E + 1)],
                            wrec_sb[:, (g * KC + k) * 128:(g * KC + k + 1) * 128],
                            start=(k == 0), stop=(k == KC - 1),
                        )
                    nc.vector.tensor_copy(
                        hwr_sb[:, (g * 4 + E) * 128:(g * 4 + E + 1) * 128], ps3[:, :]
                    )

        # ---------- state & per-step pools ----------
        sp = ctx.enter_context(tc.tile_pool(name="step", bufs=2))
        psp = ctx.enter_context(tc.tile_pool(name="spsum", bufs=1, space="PSUM"))

        chain = [None]

        def preps(pay, recs, rsem, lsem):
            prev = chain[0]
            for k in range(1, N_CORES):
                rdests = [None] * N_CORES
                rdests[k] = (0, k)
                inst = nc.gpsimd.remote_dma_broadcast(
                    out_ap=recs(k), in_ap=pay, remote_sem=rsem, local_sem=lsem,
                    rdests=rdests,
                )
                if prev is not None:
                    add_dep_helper(inst.ins, prev, False, "swdge ring order")
                prev = inst.ins
            chain[0] = prev

        npay = [0]
        lastfence = [None]

        def trig(payload_insts):
            fence = nc.vector.sem_inc(paysem, 1)
            for pi in payload_insts:
                add_dep_helper(fence.ins, pi.ins, False, "fence after payload")
            if lastfence[0] is not None:
                add_dep_helper(fence.ins, lastfence[0].ins, False, "fence order")
            lastfence[0] = fence
            npay[0] += 1
            w = nc.gpsimd.wait_ge(paysem, npay[0])
            add_dep_helper(w.ins, chain[0], False, "swdge ring order")
            tg = nc.gpsimd.trigger_dma(count=None)
            add_dep_helper(tg.ins, w.ins, False, "swdge ring order")
            chain[0] = tg.ins

        def hc_prep(r, p):
            d = 1 << r
            rdests = [None] * N_CORES
            rdests[d] = (0, d)
            src = pay1[p] if r == 0 else sbuf[r - 1][p]
            inst = nc.gpsimd.remote_dma_broadcast(
                out_ap=recr[r][p][:, 0:16], in_ap=src[:, 0:16],
                remote_sem=semr[r], local_sem=lsemr[r], rdests=rdests,
            )
            if chain[0] is not None:
                add_dep_helper(inst.ins, chain[0], False, "swdge ring order")
            chain[0] = inst.ins

        def hc_trig(r, payload_insts, t):
            fence = nc.vector.sem_inc(fsem[r], 1)
            for pi in payload_insts:
                add_dep_helper(fence.ins, pi.ins, False, "fence after payload")
            if lastfence[0] is not None:
                add_dep_helper(fence.ins, lastfence[0].ins, False, "fence order")
            lastfence[0] = fence
            w = nc.gpsimd.wait_ge(fsem[r], t + 1)
            add_dep_helper(w.ins, chain[0], False, "swdge ring order")
            tg = nc.gpsimd.trigger_dma(count=None)
            add_dep_helper(tg.ins, w.ins, False, "swdge ring order")
            chain[0] = tg.ins

        gates3 = comm.tile([128, 1], F32, tag="gates3")
        ccol = sp.tile([128, 1], F32, tag="ccol")
        nc.vector.memset(ccol[:, :], 0.0)
        a1sb_prev = None
        rz_prev = None
        psA2 = None
        psA1 = psp.tile([128, 4], F32, tag="psA1")

        for t in range(TR):
            par = t & 1

            if t == 0 and not no_comm:
                if hc:
                    hc_prep(0, 0)
                else:
                    preps(pay1[0][:, 0:16],
                          lambda k: rec1[0][:, (k - 1) * 16:k * 16], sem1, lsem1)

            # ===== gates(t) =====
            gates = sp.tile([128, 4], F32, tag="gates")
            if t == 0:
                nc.scalar.activation(gates[:, 0:3], xprojt_sb[:, 4 * t:4 * t + 3], AF.Sigmoid)
                nc.scalar.activation(gates3[:, :], xprojt_sb[:, 4 * t + 3:4 * t + 4], AF.Tanh)
            else:
                dpre = sp.tile([128, 3], F32, tag="dpre")
                nc.vector.scalar_tensor_tensor(
                    dpre[:, :], psA2[:, 0:3], rz_prev[:, :], a1sb_prev[:, 0:3],
                    mybir.AluOpType.mult, mybir.AluOpType.add)
                nc.scalar.activation(gates[:, 0:3], dpre[:, :], AF.Sigmoid)
            iu = sp.tile([128, 1], F32, tag="iu")
            nc.vector.tensor_mul(iu[:, :], gates[:, 0:1], gates3[:, :])
            ccol_new = sp.tile([128, 1], F32, tag="ccol")
            nc.vector.scalar_tensor_tensor(
                ccol_new[:, :], ccol[:, :], gates[:, 2:3], iu[:, :],
                mybir.AluOpType.mult, mybir.AluOpType.add)
            ccol = ccol_new
            tanh_c = sp.tile([128, 1], F32, tag="tanh_c")
            nc.scalar.activation(tanh_c[:, :], ccol[:, :], AF.Tanh)
            h_new = sp.tile([128, 1], F32, tag="h_new")
            hn_inst = nc.vector.tensor_mul(h_new[:, :], gates[:, 1:2], tanh_c[:, :])

            # ===== pay1 = [h_new one-hot | W1^T h_new] ; TRIG =====
            psW = psp.tile([128, KC], F32, tag="psW")
            for m in range(KC):
                nc.tensor.matmul(
                    psW[:, m:m + 1], w1t_sb[:, m * 128:(m + 1) * 128], h_new[:, :],
                    start=True, stop=True,
                )
            if t >= 2 and not no_comm:
                lw1 = nc.vector.wait_ge(lsemr[0] if hc else lsem1,
                                        16 * (t - 1) if hc else 112 * t)
                add_dep_helper(lw1.ins, hn_inst.ins, False, "anchor lsem1 wait")
                add_dep_helper(lw1.ins, lastfence[0].ins, False, "wait after fence")
            mm1 = nc.vector.tensor_scalar_mul(pay1[par][:, 0:8], mask_sb[:, :], h_new[:, :])
            cp1 = nc.vector.tensor_copy(pay1[par][:, 8:16], psW[:, :])
            if t >= 2 and not no_comm:
                add_dep_helper(mm1.ins, lw1.ins, False, "pay1 WAR")
                add_dep_helper(cp1.ins, lw1.ins, False, "pay1 WAR")
            red1 = sp.tile([128, 16], F32, tag="red1")
            if not no_comm and hc:
                # --- 3-round XOR hypercube allreduce ---
                hc_trig(0, [mm1, cp1], t)
                hc_prep(1, par)
                w0 = nc.vector.wait_ge(semr[0], 2 * (t + 1))
                add_dep_helper(w0.ins, cp1.ins, False, "anchor")
                add_dep_helper(w0.ins, lastfence[0].ins, False, "after fence")
                if t >= 2:
                    lw1r = nc.vector.wait_ge(lsemr[1], 16 * (t - 1))
                    add_dep_helper(lw1r.ins, w0.ins, False, "order")
                a1i = nc.vector.tensor_add(sbuf[0][par][:, :], pay1[par][:, :],
                                           recr[0][par][:, :])
                add_dep_helper(a1i.ins, w0.ins, False, "gate recv r0")
                if t >= 2:
                    add_dep_helper(a1i.ins, lw1r.ins, False, "s1 WAR")
                hc_trig(1, [a1i], t)
                hc_prep(2, par)
                w1r = nc.vector.wait_ge(semr[1], 2 * (t + 1))
                add_dep_helper(w1r.ins, a1i.ins, False, "anchor")
                add_dep_helper(w1r.ins, lastfence[0].ins, False, "after fence")
                if t >= 2:
                    lw2r = nc.vector.wait_ge(lsemr[2], 16 * (t - 1))
                    add_dep_helper(lw2r.ins, w1r.ins, False, "order")
                a2i = nc.vector.tensor_add(sbuf[1][par][:, :], sbuf[0][par][:, :],
                                           recr[1][par][:, :])
                add_dep_helper(a2i.ins, w1r.ins, False, "gate recv r1")
                if t >= 2:
                    add_dep_helper(a2i.ins, lw2r.ins, False, "s2 WAR")
                hc_trig(2, [a2i], t)
                if t + 1 < TR:
                    hc_prep(0, 1 - par)
                w2r = nc.vector.wait_ge(semr[2], 2 * (t + 1))
                add_dep_helper(w2r.ins, a2i.ins, False, "anchor")
                add_dep_helper(w2r.ins, lastfence[0].ins, False, "after fence")
                # final combine: w half first (attention consumes it first)
                iw = nc.vector.tensor_add(red1[:, 8:16], sbuf[1][par][:, 8:16],
                                          recr[2][par][:, 8:16])
                ihh = nc.vector.tensor_add(red1[:, 0:8], sbuf[1][par][:, 0:8],
                                           recr[2][par][:, 0:8])
                add_dep_helper(iw.ins, w2r.ins, False, "gate recv r2")
                add_dep_helper(ihh.ins, w2r.ins, False, "gate recv r2")
            else:
                if not no_comm:
                    trig([mm1, cp1])
                    if t + 1 < TR:
                        preps(pay1[1 - par][:, 0:16],
                              lambda k: rec1[1 - par][:, (k - 1) * 16:k * 16], sem1, lsem1)
                    if not free_run:
                        w1_inst = nc.vector.wait_ge(sem1, 14 * (t + 1))
                        add_dep_helper(w1_inst.ins, cp1.ins, False, "anchor sem1 wait")
                        add_dep_helper(w1_inst.ins, lastfence[0].ins, False, "wait after fence")

                r1 = rec1[par][:, :].rearrange("p (s c) -> p c s", s=N_CORES - 1)
                i1 = nc.vector.reduce_sum(red1[:, 8:16], r1[:, 8:16, :], axis=mybir.AxisListType.X)
                nc.vector.tensor_add(red1[:, 8:16], red1[:, 8:16], pay1[par][:, 8:16])
                ih = nc.vector.reduce_sum(red1[:, 0:8], r1[:, 0:8, :], axis=mybir.AxisListType.X)
                nc.vector.tensor_add(red1[:, 0:8], red1[:, 0:8], pay1[par][:, 0:8])
                if not no_comm and not free_run:
                    add_dep_helper(i1.ins, w1_inst.ins, False, "gate recv1 on sem1")
                    add_dep_helper(ih.ins, w1_inst.ins, False, "gate recv1 on sem1")

            # ===== psA1(t+1) = Wr^T (csum + hnew_full): PE, overlaps ACT =====
            a1sb = None
            if t + 1 < TR:
                if bf16:
                    red1u = sp.tile([128, KC], wdt, tag="red1u")
                    nc.vector.tensor_copy(red1u[:, :], red1[:, 0:8])
                for g in range(4):
                    for k in range(KC):
                        nc.tensor.matmul(
                            psA1[:, g:g + 1],
                            wrec_w[:, (g * KC + k) * 128:(g * KC + k + 1) * 128],
                            red1u[:, k:k + 1] if bf16 else red1[:, k:k + 1],
                            start=(k == 0), stop=(k == KC - 1),
                        )
                a1sb = sp.tile([128, 4], F32, tag="a1sb")
                nc.vector.tensor_add(a1sb[:, :], psA1[:, :],
                                     xprojt_sb[:, 4 * (t + 1):4 * (t + 1) + 4])

            # ===== attention (replicated, all 512 rows) =====
            tT = sp.tile([128, KC * MROWS], F32, tag="tT")
            psS = psp.tile([128, 4], F32, tag="psS")
            for c in range(KC):
                nc.scalar.activation(
                    tT[:, c * MROWS:(c + 1) * MROWS],
                    hw2T_sb[:, c * MROWS:(c + 1) * MROWS],
                    AF.Tanh, bias=red1[:, 8 + c:9 + c],
                )
                for r in range(4):
                    nc.tensor.matmul(
                        psS[:, r:r + 1],
                        tT[:, c * MROWS + 128 * r:c * MROWS + 128 * (r + 1)],
                        wa_sb[:, c:c + 1],
                        start=(c == 0), stop=(c == KC - 1),
                    )
            ep = sp.tile([128, 4], F32, tag="ep")
            nc.scalar.activation(ep[:, :], psS[:, :], AF.Sigmoid)
            en = sp.tile([128, 4], F32, tag="en")
            nc.scalar.activation(en[:, :], psS[:, :], AF.Sigmoid, scale=-1.0)
            ren = sp.tile([128, 4], F32, tag="ren")
            nc.vector.reciprocal(ren[:, :], en[:, :])
            e4 = sp.tile([128, 4], F32, tag="e4")
            nc.vector.tensor_mul(e4[:, :], ep[:, :], ren[:, :])

            # Z and 1/Z (psZ ahead of A2 in the PE FIFO)
            zpart = sp.tile([128, 1], F32, tag="zpart")
            nc.vector.reduce_sum(zpart[:, :], e4[:, :], axis=mybir.AxisListType.X)
            psZ = psp.tile([128, 1], F32, tag="psZ")
            nc.tensor.matmul(psZ[:, :], ones_sb[:, :], zpart[:, :], start=True, stop=True)
            rz = sp.tile([128, 1], F32, tag="rz")
            nc.vector.reciprocal(rz[:, :], psZ[:, :])

            if bf16:
                e4u = sp.tile([128, 4], wdt, tag="e4u")
                nc.vector.tensor_copy(e4u[:, :], e4[:, :])
            else:
                e4u = e4
            psA2 = psp.tile([128, 4], F32, tag="psA2")
            for g in (3, 0, 1, 2):
                for E in range(4):
                    nc.tensor.matmul(
                        psA2[:, g:g + 1],
                        hwr_sb[:, (g * 4 + E) * 128:(g * 4 + E + 1) * 128],
                        e4u[:, E:E + 1],
                        start=(E == 0), stop=(E == 3),
                    )
                if g == 3 and t + 1 < TR:
                    dpre3 = sp.tile([128, 1], F32, tag="dpre3")
                    nc.vector.scalar_tensor_tensor(
                        dpre3[:, :], psA2[:, 3:4], rz[:, :], a1sb[:, 3:4],
                        mybir.AluOpType.mult, mybir.AluOpType.add)
                    nc.scalar.activation(gates3[:, :], dpre3[:, :], AF.Tanh)

            # own slice of numer + h_att(t) output (off critical path)
            psNo = psp.tile([128, 1], F32, tag="psNo")
            for E in range(4):
                nc.tensor.matmul(
                    psNo[:, :], hTj_w[:, 128 * E:128 * (E + 1)], e4u[:, E:E + 1],
                    start=(E == 0), stop=(E == 3),
                )
            htmp = sp.tile([128, 1], F32, tag="htmp")
            nc.vector.tensor_add(htmp[:, :], h_new[:, :], csumj_sb[:, :])
            nc.vector.scalar_tensor_tensor(
                houtbuf[:, t:t + 1], psNo[:, :], rz[:, :], htmp[:, :],
                mybir.AluOpType.mult, mybir.AluOpType.add)

            a1sb_prev = a1sb
            rz_prev = rz

        nc.sync.dma_start(hout.ap(), houtbuf[:, :])

    nc.compile()
    return nc


# ============================================================================
# v4: v3 compute + HUB-REDUCE sync. Leaves (cores 1-7) each send pay [128,16]
# to core 0's hrec slot with ONE SWDGE prep; the hub reduces 7 slots + its own
# pay into red1b and broadcasts it back with ONE multi-dest prep (a broadcast
# prep costs the same 66 descs regardless of fan-out). Pool work per core per
# step drops from 7 preps (~46us) to 1 prep (~7us). Leaves skip the reduce.
# Comm code runs in tc.If(pid==...) branches; triggers use explicit count=1
# gated on the prep's own .then_inc sem. Gather on SWDGE queue 0, broadcast
# on queue 1 (separate FIFOs).
# ============================================================================
def build_nc_v4(T: int, t_run: int | None = None, no_comm: bool = False,
                bf16: bool = False, bq: int = 1):
    del _EXTERNAL_SEMS[:]
    wdt = mybir.dt.bfloat16 if bf16 else F32
    nc = bacc.Bacc(dynamic_dma_scratch_size=32768, num_swdge_queues=2)
    TR = t_run if t_run is not None else T

    dp = lambda n, s_, dt=F32: nc.declare_dram_parameter(n, s_, dt, isOutput=False)
    xT = dp("xT", [128, KC * T])
    wx = dp("wx", [128, 4 * KC * 128])
    wrec = dp("wrec", [128, 4 * KC * 128])
    w1t = dp("w1t", [128, KC * 128])
    w2t = dp("w2t", [128, KC * KC * 128])
    hTs = dp("hT", [128, KC * MROWS])
    hTj = dp("hTj", [128, MROWS])
    wa = dp("wa", [128, KC])
    bias_x = dp("bias_x", [128, 4])
    bias2 = dp("bias2", [128, KC])
    mask = dp("mask", [128, KC])
    hout = nc.declare_dram_parameter("hout", [128, T], F32, isOutput=True)

    with tile.TileContext(nc) as tc, ExitStack() as ctx:
        paysem = ctx.enter_context(nc.semaphore("paysem"))
        hubsem = ctx.enter_context(nc.semaphore("hubsem"))
        sem_g = ctx.enter_context(nc.semaphore("rdma_sem_g"))
        sem_b = ctx.enter_context(nc.semaphore("rdma_sem_b"))
        lsem_g = ctx.enter_context(nc.semaphore("rdma_lsem_g"))
        lsem_b = ctx.enter_context(nc.semaphore("rdma_lsem_b"))
        # paysem/hubsem increments live in different basic blocks than their
        # waits (If branches); Tile's block-local scheduling sim can't see
        # cross-block sem state, so pre-satisfy them there too. The runtime
        # NEFF keeps the real waits.
        _EXTERNAL_SEMS.extend([sem_g, sem_b, lsem_g, lsem_b, paysem, hubsem])

        pid = nc.partition_id()

        comm = ctx.enter_context(tc.tile_pool(name="comm", bufs=1))
        pay1 = [comm.tile([128, 16], F32, name=f"pay1_{p}", tag=f"pay1_{p}") for p in range(2)]
        hrec = [comm.tile([128, 112], F32, name=f"hrec_{p}", tag=f"hrec_{p}") for p in range(2)]
        red1b = [comm.tile([128, 16], F32, name=f"red1b_{p}", tag=f"red1b_{p}") for p in range(2)]

        const = ctx.enter_context(tc.tile_pool(name="const", bufs=1))
        wrec_sb = const.tile([128, 4 * KC * 128], F32, tag="wrec")
        w1t_sb = const.tile([128, KC * 128], F32, tag="w1t")
        hTj_sb = const.tile([128, MROWS], F32, tag="hTj")
        csumj_sb = const.tile([128, 1], F32, tag="csumj")
        wa_sb = const.tile([128, KC], F32, tag="wa")
        hw2T_sb = const.tile([128, KC * MROWS], F32, tag="hw2T")
        hwr_sb = const.tile([128, 16 * 128], wdt, tag="hwr")
        xproj_sb = const.tile([128, 4 * T], F32, tag="xproj")
        xprojt_sb = const.tile([128, 4 * T], F32, tag="xprojt")
        csum_sb = const.tile([128, KC], F32, tag="csum")
        ones_sb = const.tile([128, 128], F32, tag="ones")
        mask_sb = const.tile([128, KC], F32, tag="mask")
        houtbuf = const.tile([128, T], F32, tag="houtbuf")
        if bf16:
            wrec_w = const.tile([128, 4 * KC * 128], wdt, tag="wrec_w")
            hTj_w = const.tile([128, MROWS], wdt, tag="hTj_w")

        nc.sync.dma_start(wrec_sb[:, :], wrec.ap())
        nc.sync.dma_start(w1t_sb[:, :], w1t.ap())
        nc.sync.dma_start(hTj_sb[:, :], hTj.ap())
        nc.sync.dma_start(wa_sb[:, :], wa.ap())
        nc.sync.dma_start(mask_sb[:, :], mask.ap())
        nc.vector.memset(ones_sb[:, :], -1.0)
        if bf16:
            nc.vector.tensor_copy(wrec_w[:, :], wrec_sb[:, :])
            nc.vector.tensor_copy(hTj_w[:, :], hTj_sb[:, :])
        else:
            wrec_w, hTj_w = wrec_sb, hTj_sb
        nc.vector.memset(houtbuf[:, :], 0.0)
        if no_comm:
            for p in range(2):
                nc.vector.memset(red1b[p][:, :], 0.01)
                nc.vector.memset(pay1[p][:, :], 0.01)

        # ---------- device precompute (identical to v3) ----------
        with tc.tile_pool(name="pre", bufs=1) as pre, \
             tc.tile_pool(name="prepsum", bufs=1, space="PSUM") as pps:
            xT_sb = pre.tile([128, KC * T], F32, tag="xT")
            wx_sb = pre.tile([128, 4 * KC * 128], F32, tag="wx")
            w2t_sb = pre.tile([128, KC * KC * 128], F32, tag="w2t")
            hT_sb = pre.tile([128, KC * MROWS], F32, tag="hT")
            bx_sb = pre.tile([128, 4], F32, tag="bias_x")
            b2_sb = pre.tile([128, KC], F32, tag="bias2")
            nc.sync.dma_start(xT_sb[:, :], xT.ap())
            nc.sync.dma_start(wx_sb[:, :], wx.ap())
            nc.sync.dma_start(w2t_sb[:, :], w2t.ap())
            nc.sync.dma_start(hT_sb[:, :], hTs.ap())
            nc.sync.dma_start(bx_sb[:, :], bias_x.ap())
            nc.sync.dma_start(b2_sb[:, :], bias2.ap())

            for m in range(KC):
                nc.vector.reduce_sum(
                    csum_sb[:, m:m + 1],
                    hT_sb[:, m * MROWS:(m + 1) * MROWS],
                    axis=mybir.AxisListType.X,
                )
            cm = pre.tile([128, KC], F32, tag="cmask")
            nc.vector.tensor_mul(cm[:, :], csum_sb[:, :], mask_sb[:, :])
            nc.vector.reduce_sum(csumj_sb[:, :], cm[:, :], axis=mybir.AxisListType.X)

            psg0 = pps.tile([128, 4], F32, tag="ps_g0")
            for g in range(4):
                for k in range(KC):
                    nc.tensor.matmul(
                        psg0[:, g:g + 1],
                        wrec_sb[:, (g * KC + k) * 128:(g * KC + k + 1) * 128],
                        csum_sb[:, k:k + 1],
                        start=(k == 0), stop=(k == KC - 1),
                    )
            biasc = pre.tile([128, 4], F32, tag="biasc")
            nc.vector.tensor_add(biasc[:, :], psg0[:, :], bx_sb[:, :])

            for g in range(4):
                ps = pps.tile([128, T], F32, tag="ps_x")
                for k in range(KC):
                    nc.tensor.matmul(
                        ps[:, :],
                        wx_sb[:, (g * KC + k) * 128:(g * KC + k + 1) * 128],
                        xT_sb[:, k * T:(k + 1) * T],
                        start=(k == 0), stop=(k == KC - 1),
                    )
                nc.vector.tensor_scalar_add(
                    xproj_sb[:, g * T:(g + 1) * T], ps[:, :], biasc[:, g:g + 1]
                )
                nc.vector.tensor_sub(
                    xproj_sb[:, g * T:g * T + 1],
                    xproj_sb[:, g * T:g * T + 1], psg0[:, g:g + 1]
                )
            nc.vector.tensor_copy(
                xprojt_sb[:, :].rearrange("p (t g) -> p t g", g=4),
                xproj_sb[:, :].rearrange("p (g t) -> p t g", g=4),
            )

            for c in range(KC):
                ps2 = pps.tile([128, MROWS], F32, tag="ps_h")
                for k in range(KC):
                    nc.tensor.matmul(
                        ps2[:, :],
                        w2t_sb[:, (c * KC + k) * 128:(c * KC + k + 1) * 128],
                        hT_sb[:, k * MROWS:(k + 1) * MROWS],
                        start=(k == 0), stop=(k == KC - 1),
                    )
                nc.vector.tensor_scalar_add(
                    hw2T_sb[:, c * MROWS:(c + 1) * MROWS], ps2[:, :], b2_sb[:, c:c + 1]
                )

            for g in range(4):
                for E in range(4):
                    ps3 = pps.tile([128, 128], F32, tag="ps_hwr")
                    for k in range(KC):
                        nc.tensor.matmul(
                            ps3[:, :],
                            hT_sb[:, k * MROWS + 128 * E:k * MROWS + 128 * (E + 1)],
                            wrec_sb[:, (g * KC + k) * 128:(g * KC + k + 1) * 128],
                            start=(k == 0), stop=(k == KC - 1),
                        )
                    nc.vector.tensor_copy(
                        hwr_sb[:, (g * 4 + E) * 128:(g * 4 + E + 1) * 128], ps3[:, :]
                    )

        # ---------- state & per-step pools ----------
        sp = ctx.enter_context(tc.tile_pool(name="step", bufs=2))
        psp = ctx.enter_context(tc.tile_pool(name="spsum", bufs=1, space="PSUM"))

        chain_g = [None]   # queue-0 ring order (leaf gather)
        chain_b = [None]   # queue-1 ring order (hub bcast)

        # Each prep is stashed out of the gpsimd pending list at emission and
        # restored right before ITS trigger, so trigger_dma(count=None) bakes
        # _count=1 (matching the per-core runtime ring) while still getting
        # Tile's prep-commit OnUpdate ordering (explicit count=N fires before
        # the async Q7 desc commit and corrupts the ring — measured hang).
        pending = nc.gpsimd._pending_untriggered_insts
        stash_g = [[None] * N_CORES, [None] * N_CORES]
        stash_b = [None, None]

        def prep_gather(par):
            # 7 If-branches; each leaf emits exactly one prep.
            for j in range(1, N_CORES):
                with tc.If(pid == j):
                    rdests = [None] * N_CORES
                    rdests[j] = (0, j)   # dest = own_id ^ j == 0 (the hub)
                    inst = nc.gpsimd.remote_dma_broadcast(
                        out_ap=hrec[par][:, (j - 1) * 16:j * 16],
                        in_ap=pay1[par][:, 0:16],
                        remote_sem=sem_g, local_sem=lsem_g,
                        rdests=rdests, queue_num=0,
                    )
                    pending[0].remove(inst)
                    stash_g[par][j] = inst

        def prep_bcast(par):
            with tc.If(pid == 0):
                rdests = [(0, k) if k else None for k in range(N_CORES)]
                inst = nc.gpsimd.remote_dma_broadcast(
                    out_ap=red1b[par][:, 0:16],
                    in_ap=red1b[par][:, 0:16],
                    remote_sem=sem_b, local_sem=lsem_b,
                    rdests=rdests, queue_num=0,
                )
                pending[0].remove(inst)
                stash_b[par] = inst

        lastfence = [None]
        lasthfence = [None]

        gates3 = comm.tile([128, 1], F32, tag="gates3")
        ccol = sp.tile([128, 1], F32, tag="ccol")
        nc.vector.memset(ccol[:, :], 0.0)
        a1sb_prev = None
        rz_prev = None
        psA2 = None
        psA1 = psp.tile([128, 4], F32, tag="psA1")

        for t in range(TR):
            par = t & 1

            if t == 0 and not no_comm:
                prep_gather(0)
                prep_bcast(0)

            # ===== gates(t) =====
            gates = sp.tile([128, 4], F32, tag="gates")
            if t == 0:
                nc.scalar.activation(gates[:, 0:3], xprojt_sb[:, 4 * t:4 * t + 3], AF.Sigmoid)
                nc.scalar.activation(gates3[:, :], xprojt_sb[:, 4 * t + 3:4 * t + 4], AF.Tanh)
            else:
                dpre = sp.tile([128, 3], F32, tag="dpre")
                nc.vector.scalar_tensor_tensor(
                    dpre[:, :], psA2[:, 0:3], rz_prev[:, :], a1sb_prev[:, 0:3],
                    mybir.AluOpType.mult, mybir.AluOpType.add)
                nc.scalar.activation(gates[:, 0:3], dpre[:, :], AF.Sigmoid)
            iu = sp.tile([128, 1], F32, tag="iu")
            nc.vector.tensor_mul(iu[:, :], gates[:, 0:1], gates3[:, :])
            ccol_new = sp.tile([128, 1], F32, tag="ccol")
            nc.vector.scalar_tensor_tensor(
                ccol_new[:, :], ccol[:, :], gates[:, 2:3], iu[:, :],
                mybir.AluOpType.mult, mybir.AluOpType.add)
            ccol = ccol_new
            tanh_c = sp.tile([128, 1], F32, tag="tanh_c")
            nc.scalar.activation(tanh_c[:, :], ccol[:, :], AF.Tanh)
            h_new = sp.tile([128, 1], F32, tag="h_new")
            hn_inst = nc.vector.tensor_mul(h_new[:, :], gates[:, 1:2], tanh_c[:, :])

            # ===== pay1 = [h_new one-hot | W1^T h_new] =====
            psW = psp.tile([128, KC], F32, tag="psW")
            for m in range(KC):
                nc.tensor.matmul(
                    psW[:, m:m + 1], w1t_sb[:, m * 128:(m + 1) * 128], h_new[:, :],
                    start=True, stop=True,
                )
            if t >= 2 and not no_comm:
                with tc.If(pid >= 1):
                    lwg = nc.vector.wait_ge(lsem_g, 16 * (t - 1))
                    add_dep_helper(lwg.ins, hn_inst.ins, False, "anchor lsem_g")
                    add_dep_helper(lwg.ins, lastfence[0].ins, False, "after fence")
                    # tile-visible marker: pay1[par] free for rewrite
                    mkp = nc.vector.tensor_copy(pay1[par][:, 0:1], pay1[par][:, 0:1])
                    add_dep_helper(mkp.ins, lwg.ins, False, "marker after wait")
            mm1 = nc.vector.tensor_scalar_mul(pay1[par][:, 0:8], mask_sb[:, :], h_new[:, :])
            cp1 = nc.vector.tensor_copy(pay1[par][:, 8:16], psW[:, :])

            if not no_comm:
                # fence: payload written -> leaves fire gather
                fence = nc.vector.sem_inc(paysem, 1)
                add_dep_helper(fence.ins, mm1.ins, False, "fence after payload")
                add_dep_helper(fence.ins, cp1.ins, False, "fence after payload")
                if lastfence[0] is not None:
                    add_dep_helper(fence.ins, lastfence[0].ins, False, "fence order")
                lastfence = [fence]
                for j in range(1, N_CORES):
                    with tc.If(pid == j):
                        wp = nc.gpsimd.wait_ge(paysem, t + 1)
                        pending[0][:] = [stash_g[par][j]]
                        tg = nc.gpsimd.trigger_dma(count=None, queue_num=0)
                        add_dep_helper(tg.ins, wp.ins, False, "q0 order")
                # hoist next step's gather preps (off-path, during flight)
                if t + 1 < TR:
                    prep_gather(1 - par)

                # hub: receive, reduce, fire bcast
                with tc.If(pid == 0):
                    wgr = nc.vector.wait_ge(sem_g, 14 * (t + 1))
                    add_dep_helper(wgr.ins, fence.ins, False, "after fence")
                    if t >= 2:
                        lwb = nc.vector.wait_ge(lsem_b, 16 * (t - 1))
                        add_dep_helper(lwb.ins, wgr.ins, False, "order")
                    hr = hrec[par][:, :].rearrange("p (s c) -> p c s", s=N_CORES - 1)
                    rw = nc.vector.reduce_sum(red1b[par][:, 8:16], hr[:, 8:16, :],
                                              axis=mybir.AxisListType.X)
                    aw = nc.vector.tensor_add(red1b[par][:, 8:16], red1b[par][:, 8:16],
                                              pay1[par][:, 8:16])
                    rh = nc.vector.reduce_sum(red1b[par][:, 0:8], hr[:, 0:8, :],
                                              axis=mybir.AxisListType.X)
                    ah = nc.vector.tensor_add(red1b[par][:, 0:8], red1b[par][:, 0:8],
                                              pay1[par][:, 0:8])
                    add_dep_helper(rw.ins, wgr.ins, False, "gate recv on sem_g")
                    add_dep_helper(rh.ins, wgr.ins, False, "gate recv on sem_g")
                    if t >= 2:
                        add_dep_helper(rw.ins, lwb.ins, False, "red1b WAR")
                        add_dep_helper(rh.ins, lwb.ins, False, "red1b WAR")
                    hfence = nc.vector.sem_inc(hubsem, 1)
                    add_dep_helper(hfence.ins, aw.ins, False, "after reduce")
                    add_dep_helper(hfence.ins, ah.ins, False, "after reduce")
                    if lasthfence[0] is not None:
                        add_dep_helper(hfence.ins, lasthfence[0].ins, False, "order")
                    lasthfence = [hfence]
                    wh = nc.gpsimd.wait_ge(hubsem, t + 1)
                    pending[0][:] = [stash_b[par]]
                    tb = nc.gpsimd.trigger_dma(count=None, queue_num=0)
                    add_dep_helper(tb.ins, wh.ins, False, "q1 order")
                if t + 1 < TR:
                    prep_bcast(1 - par)

                # leaves: wait for the reduced result; marker-write red1b so
                # shared consumers order after the branch join.
                with tc.If(pid >= 1):
                    wbr = nc.vector.wait_ge(sem_b, 2 * (t + 1))
                    add_dep_helper(wbr.ins, cp1.ins, False, "anchor")
                    add_dep_helper(wbr.ins, lastfence[0].ins, False, "after fence")
                    mkr = nc.vector.tensor_copy(red1b[par][:, 0:1], red1b[par][:, 0:1])
                    add_dep_helper(mkr.ins, wbr.ins, False, "marker after wait")

            red1 = red1b[par]

            # ===== psA1(t+1) = Wr^T (csum + hnew_full): PE, overlaps ACT =====
            a1sb = None
            if t + 1 < TR:
                if bf16:
                    red1u = sp.tile([128, KC], wdt, tag="red1u")
                    nc.vector.tensor_copy(red1u[:, :], red1[:, 0:8])
                for g in range(4):
                    for k in range(KC):
                        nc.tensor.matmul(
                            psA1[:, g:g + 1],
                            wrec_w[:, (g * KC + k) * 128:(g * KC + k + 1) * 128],
                            red1u[:, k:k + 1] if bf16 else red1[:, k:k + 1],
                            start=(k == 0), stop=(k == KC - 1),
                        )
                a1sb = sp.tile([128, 4], F32, tag="a1sb")
                nc.vector.tensor_add(a1sb[:, :], psA1[:, :],
                                     xprojt_sb[:, 4 * (t + 1):4 * (t + 1) + 4])

            # ===== attention (replicated, all 512 rows) =====
            tT = sp.tile([128, KC * MROWS], F32, tag="tT")
            psS = psp.tile([128, 4], F32, tag="psS")
            for c in range(KC):
                nc.scalar.activation(
                    tT[:, c * MROWS:(c + 1) * MROWS],
                    hw2T_sb[:, c * MROWS:(c + 1) * MROWS],
                    AF.Tanh, bias=red1[:, 8 + c:9 + c],
                )
                for r in range(4):
                    nc.tensor.matmul(
                        psS[:, r:r + 1],
                        tT[:, c * MROWS + 128 * r:c * MROWS + 128 * (r + 1)],
                        wa_sb[:, c:c + 1],
                        start=(c == 0), stop=(c == KC - 1),
                    )
            ep = sp.tile([128, 4], F32, tag="ep")
            nc.scalar.activation(ep[:, :], psS[:, :], AF.Sigmoid)
            en = sp.tile([128, 4], F32, tag="en")
            nc.scalar.activation(en[:, :], psS[:, :], AF.Sigmoid, scale=-1.0)
            ren = sp.tile([128, 4], F32, tag="ren")
            nc.vector.reciprocal(ren[:, :], en[:, :])
            e4 = sp.tile([128, 4], F32, tag="e4")
            nc.vector.tensor_mul(e4[:, :], ep[:, :], ren[:, :])

            zpart = sp.tile([128, 1], F32, tag="zpart")
            nc.vector.reduce_sum(zpart[:, :], e4[:, :], axis=mybir.AxisListType.X)
            psZ = psp.tile([128, 1], F32, tag="psZ")
            nc.tensor.matmul(psZ[:, :], ones_sb[:, :], zpart[:, :], start=True, stop=True)
            rz = sp.tile([128, 1], F32, tag="rz")
            nc.vector.reciprocal(rz[:, :], psZ[:, :])

            if bf16:
                e4u = sp.tile([128, 4], wdt, tag="e4u")
                nc.vector.tensor_copy(e4u[:, :], e4[:, :])
            else:
                e4u = e4
            psA2 = psp.tile([128, 4], F32, tag="psA2")
            for g in (3, 0, 1, 2):
                for E in range(4):
                    nc.tensor.matmul(
                        psA2[:, g:g + 1],
                        hwr_sb[:, (g * 4 + E) * 128:(g * 4 + E + 1) * 128],
                        e4u[:, E:E + 1],
                        start=(E == 0), stop=(E == 3),
                    )
                if g == 3 and t + 1 < TR:
                    dpre3 = sp.tile([128, 1], F32, tag="dpre3")
                    nc.vector.scalar_tensor_tensor(
                        dpre3[:, :], psA2[:, 3:4], rz[:, :], a1sb[:, 3:4],
                        mybir.AluOpType.mult, mybir.AluOpType.add)
                    nc.scalar.activation(gates3[:, :], dpre3[:, :], AF.Tanh)

            psNo = psp.tile([128, 1], F32, tag="psNo")
            for E in range(4):
                nc.tensor.matmul(
                    psNo[:, :], hTj_w[:, 128 * E:128 * (E + 1)], e4u[:, E:E + 1],
                    start=(E == 0), stop=(E == 3),
                )
            htmp = sp.tile([128, 1], F32, tag="htmp")
            nc.vector.tensor_add(htmp[:, :], h_new[:, :], csumj_sb[:, :])
            nc.vector.scalar_tensor_tensor(
                houtbuf[:, t:t + 1], psNo[:, :], rz[:, :], htmp[:, :],
                mybir.AluOpType.mult, mybir.AluOpType.add)

            a1sb_prev = a1sb
            rz_prev = rz

        nc.sync.dma_start(hout.ap(), houtbuf[:, :])

    nc.compile()
    return nc


def prep_in_maps_v3(inputs: dict, T: int) -> list[dict]:
    maps = prep_in_maps_v2(inputs, T)
    for m in maps:
        m.pop("emask", None)
        m.pop("dup2", None)
        m.pop("hTown", None)
    return maps


def prep_in_maps_v2(inputs: dict, T: int) -> list[dict]:
    maps = prep_in_maps(inputs, T)
    dup2 = np.concatenate([np.eye(RPC, dtype=np.float32)] * 2, axis=1)  # [64, 128]
    for j, m in enumerate(maps):
        emask = np.zeros((128, 4), np.float32)
        emask[(j % 2) * RPC:(j % 2 + 1) * RPC, j // 2] = 1.0
        m["emask"] = emask
        m["dup2"] = np.ascontiguousarray(dup2)
        m.pop("hrows")
        # v2 uses plain f32 for these
        W_iouh = np.asarray(inputs["W_iouh"], np.float32)
        W_fh = np.asarray(inputs["W_fh"], np.float32)
        W_attnh = np.asarray(inputs["W_attnh"], np.float32)
        H = np.asarray(inputs["hiddn_state_mat"], np.float32)
        # v2 gate order [i, o, f, u]
        gate_w = [W_iouh[:, 0:MEM], W_iouh[:, MEM:2 * MEM], W_fh, W_iouh[:, 2 * MEM:]]
        W_ioux = np.asarray(inputs["W_ioux"], np.float32)
        W_fx = np.asarray(inputs["W_fx"], np.float32)
        b_iou = (np.asarray(inputs["b_ioux"], np.float32)
                 + np.asarray(inputs["b_iouh"], np.float32))
        b_f = (np.asarray(inputs["b_fx"], np.float32)
               + np.asarray(inputs["b_fh"], np.float32))
        gate_wx = [W_ioux[:, 0:MEM], W_ioux[:, MEM:2 * MEM], W_fx, W_ioux[:, 2 * MEM:]]
        gate_b = [b_iou[0:MEM], b_iou[MEM:2 * MEM], b_f, b_iou[2 * MEM:]]
        wrec = np.zeros((128, 4 * KC * 128), np.float32)
        wx = np.zeros((128, 4 * KC * 128), np.float32)
        for g in range(4):
            for k in range(KC):
                sl = np.s_[:, (g * KC + k) * 128:(g * KC + k + 1) * 128]
                wrec[sl] = gate_w[g][128 * k:128 * (k + 1), 128 * j:128 * (j + 1)]
                wx[sl] = gate_wx[g][128 * k:128 * (k + 1), 128 * j:128 * (j + 1)]
        m["wrec"] = wrec
        m["wx"] = wx
        m["bias_x"] = np.ascontiguousarray(
            np.stack([gate_b[g][128 * j:128 * (j + 1)] for g in range(4)], axis=1))
        m["hTj"] = np.ascontiguousarray(np.concatenate(
            [H[128 * E:128 * (E + 1), 128 * j:128 * (j + 1)] for E in range(4)],
            axis=1))
        W1 = W_attnh[:MEM]
        w1t = np.zeros((128, KC * 128), np.float32)
        for mm in range(KC):
            w1t[:, mm * 128:(mm + 1) * 128] = \
                W1[128 * j:128 * (j + 1), 128 * mm:128 * (mm + 1)]
        m["w1t"] = w1t
    return maps


def postprocess_v2(houts: list, T: int) -> np.ndarray:
    # per-core hout [128, T]: core j owns mem dims 128j..128j+127
    buf = np.stack([np.asarray(h).reshape(128, T) for h in houts], axis=0)  # [8,128,T]
    return np.ascontiguousarray(buf.transpose(2, 0, 1).reshape(T, MEM)).astype(np.float32)


# ----------------------------------------------------------------------------
# Harness entry point: full (unsharded) inputs -> full output.
# ----------------------------------------------------------------------------
KERNEL_BF16 = True


def kernel(**inputs) -> np.ndarray:
    from concourse.bass_utils import run_bass_kernel_spmd

    T = int(np.asarray(inputs["inputs"]).shape[0])
    nc = build_nc_v3(T, bf16=KERNEL_BF16, hc=True)
    in_maps = prep_in_maps_v3(inputs, T)
    res = run_bass_kernel_spmd(nc, in_maps, core_ids=list(range(N_CORES)))
    return postprocess_v2([res.results[c]["hout"] for c in range(N_CORES)], T)

